# revision 1
# baseline (speedup 1.0000x reference)
"""Multi-head attention block (12 heads, N=2048, C=768) on 8 NeuronCores.

Sharding: core i = (batch b = i//2, head-group g = i%2). Each core computes
attention for 6 heads of one batch plus its slice of the output projection
(row-sharded Wproj); the host sums the two head-group partials per batch.

Per-core dataflow:
  QKV projection and output projection run in float32r (full-rate fp32,
  ~1.6e-4 matmul error). Attention (scores / exp / attn@V) runs in bf16.
  xT [768,2048] arrives host-transposed; QT/KT [384,2048] are column-major
  (head h lives at partitions (h%2)*64..+64 of tile h//2), V2 is token-major
  with a ones column per head (66th col = pad for even free size).

  Heads are processed in pairs (a=2j at PE rows 0-63, b=2j+1 at rows 64-127).
  Per (pair, 512-query chunk qs, key block k):
    S^T_a -> pss[:, 0:512], S^T_b -> pss[:, 512:1024]  (two matmuls in
      disjoint PE row groups, sharing one PSUM tile so the second has no
      semaphore wait and the pair runs concurrently in the array)
    es = exp(S/8) for both heads in ONE ACT instruction (PSUM->SBUF, bf16)
    U'_a += V2_a[k]^T @ es[:, 0:512], U'_b += V2_b[k]^T @ es[:, 512:1024]
      (PSUM [66,512] accumulated over k; row 64 = softmax denominator via
      the ones column; software-pipelined one k behind the scores)
  U rows are scaled by 1/denominator (DVE reciprocal + gpsimd partition
  broadcast + fused DVE multiply) into UT [384,2048] (f32r); odd heads take
  a small DMA hop to land at partitions 64-127.
  out = UT^T-chunks @ Wproj_rows (f32r, PSUM-accumulated), DMA out.
"""

import numpy as np
from contextlib import ExitStack

import concourse.bass as bass
import concourse.tile as tile
from concourse import bacc, mybir
from concourse.bass_utils import run_bass_kernel_spmd

N_CORES = 8
C = 768          # model dim
HG = 6           # heads per core
D = 64           # head dim
CHG = HG * D     # 384, per-group qkv width
CC = C // 128    # 6 contraction chunks
MT = CHG // 128  # 3 m-tiles for QT/KT
SCALE = 1.0 / 8.0

F32 = mybir.dt.float32
F32R = mybir.dt.float32r
BF16 = mybir.dt.bfloat16


def build(n_tok: int = 2048):
    NT = n_tok
    KB = NT // 128           # key blocks
    NQ = NT // 512           # 512-wide query chunks
    EXPF = mybir.ActivationFunctionType.Exp

    nc = bacc.Bacc("TRN2", target_bir_lowering=False, debug=False,
                   num_devices=N_CORES)

    xT = nc.dram_tensor("xT", [C, NT], F32R, kind="ExternalInput").ap()
    wq = nc.dram_tensor("wq", [C, CHG], F32R, kind="ExternalInput").ap()
    wk = nc.dram_tensor("wk", [C, CHG], F32R, kind="ExternalInput").ap()
    wv = nc.dram_tensor("wv", [C, CHG], F32R, kind="ExternalInput").ap()
    wp = nc.dram_tensor("wp", [CHG, C], F32R, kind="ExternalInput").ap()
    bqk = nc.dram_tensor("bqk", [128, 2 * MT], F32, kind="ExternalInput").ap()
    bv = nc.dram_tensor("bv", [1, CHG], F32, kind="ExternalInput").ap()
    out = nc.dram_tensor("out", [NT, C], F32, kind="ExternalOutput").ap()

    with tile.TileContext(nc) as tc, ExitStack() as ctx:
        wpool = ctx.enter_context(tc.tile_pool(name="w", bufs=1))
        perm = ctx.enter_context(tc.tile_pool(name="perm", bufs=1))
        psum = ctx.enter_context(tc.tile_pool(name="ps", bufs=2, space="PSUM"))
        psum_u = ctx.enter_context(tc.tile_pool(name="psu", bufs=4,
                                                space="PSUM"))

        # ---- persistent SBUF ----
        wq_t = [wpool.tile([128, CHG], F32R, tag=f"wq{c}", name=f"wq{c}")
                for c in range(CC)]
        wk_t = [wpool.tile([128, CHG], F32R, tag=f"wk{c}", name=f"wk{c}")
                for c in range(CC)]
        wv_t = [wpool.tile([128, CHG], F32R, tag=f"wv{c}", name=f"wv{c}")
                for c in range(CC)]
        wp_t = [wpool.tile([128, C], F32R, tag=f"wp{m}", name=f"wp{m}")
                for m in range(MT)]
        bqk_t = wpool.tile([128, 2 * MT], F32, tag="bqk")
        bv_row = wpool.tile([1, CHG], F32, tag="bvr")
        bv_bc = wpool.tile([128, CHG], F32, tag="bvb")

        QT = [perm.tile([128, NT], BF16, tag=f"qt{m}", name=f"qtt{m}")
              for m in range(MT)]
        KT = [perm.tile([128, NT], BF16, tag=f"kt{m}", name=f"ktt{m}")
              for m in range(MT)]
        V2 = [perm.tile([128, HG, 66], BF16, tag=f"v2{t}", name=f"v2t{t}")
              for t in range(KB)]
        UT = [perm.tile([128, NT], F32R, tag=f"ut{m}", name=f"utt{m}")
              for m in range(MT)]

        # ---- input DMA ----
        for c in range(CC):
            nc.sync.dma_start(wq_t[c][:], wq[c * 128:(c + 1) * 128, :])
            nc.sync.dma_start(wk_t[c][:], wk[c * 128:(c + 1) * 128, :])
            nc.sync.dma_start(wv_t[c][:], wv[c * 128:(c + 1) * 128, :])
        for m in range(MT):
            nc.sync.dma_start(wp_t[m][:], wp[m * 128:(m + 1) * 128, :])
        nc.sync.dma_start(bqk_t[:], bqk)
        nc.sync.dma_start(bv_row[0:1, :], bv[0:1, :])
        nc.gpsimd.partition_broadcast(bv_bc[:], bv_row[0:1, :])
        for t in range(KB):
            nc.vector.tensor_scalar(
                V2[t][:, :, 64:66],
                bv_bc[:, 0:12].rearrange("p (a b) -> p a b", a=HG),
                0.0, 1.0, mybir.AluOpType.mult, mybir.AluOpType.add)

        spool = ctx.enter_context(tc.tile_pool(name="es", bufs=14))
        rpool = ctx.enter_context(tc.tile_pool(name="rb", bufs=4))
        stpool = ctx.enter_context(tc.tile_pool(name="st", bufs=3))
        opool = ctx.enter_context(tc.tile_pool(name="ost", bufs=3))
        xpool = ctx.enter_context(tc.tile_pool(name="xt", bufs=1))

        # ---- QKV projection pieces ----
        xt = []
        for c in range(CC):
            xc = xpool.tile([128, NT], F32R, tag=f"x{c}", name=f"xt{c}")
            nc.sync.dma_start(xc[:], xT[c * 128:(c + 1) * 128, :])
            xt.append(xc)

        def qk_mtile(m):
            for wt, dst, bcol in ((wq_t, QT, m), (wk_t, KT, MT + m)):
                for n in range(NQ):
                    ps = psum.tile([128, 512], F32, tag="ps",
                                   name=f"psqk{m}_{n}")
                    for c in range(CC):
                        nc.tensor.matmul(
                            ps[:], wt[c][:, m * 128:(m + 1) * 128],
                            xt[c][:, n * 512:(n + 1) * 512],
                            start=(c == 0), stop=(c == CC - 1))
                    nc.vector.tensor_scalar_add(
                        dst[m][:, n * 512:(n + 1) * 512], ps[:],
                        bqk_t[:, bcol:bcol + 1])

        def v_tile(t):
            ps = psum_u.tile([128, CHG], F32, tag="psu", name=f"psv{t}")
            for c in range(CC):
                nc.tensor.matmul(ps[:], xt[c][:, t * 128:(t + 1) * 128],
                                 wv_t[c][:],
                                 start=(c == 0), stop=(c == CC - 1))
            nc.vector.tensor_add(
                V2[t][:, :, 0:64],
                ps[:].rearrange("p (h d) -> p h d", h=HG),
                bv_bc[:].rearrange("p (h d) -> p h d", h=HG))

        def v_proj(ts):
            for t in ts:
                v_tile(t)

        # ---- attention pieces ----
        def attn_pair(qs, j, k_hook=None):
            ha, hb = 2 * j, 2 * j + 1
            q0 = qs * 512
            psu = [psum_u.tile([128, 512], F32, tag="psu",
                               name=f"psu{j}_{qs}_{i}") for i in range(2)]

            def emit_pv(k, es):
                for i, h in enumerate((ha, hb)):
                    nc.tensor.matmul(
                        psu[i][0:66, :], V2[k][:, h, :],
                        es[:, i * 512:(i + 1) * 512],
                        start=(k == 0), stop=(k == KB - 1))

            prev = None
            for k in range(KB):
                if k_hook is not None:
                    k_hook(k)
                pss = psum.tile([128, 1024], F32, tag="ps",
                                name=f"pss{j}_{qs}_{k}")
                # head a in PE rows 0-63 -> pss cols 0-511, head b in
                # rows 64-127 -> cols 512-1023; one shared PSUM tile so
                # the second matmul has no semaphore wait and the pair
                # runs concurrently in disjoint row groups.
                for i, off in ((0, 0), (1, 64)):
                    nc.tensor.matmul(
                        pss[:, i * 512:(i + 1) * 512],
                        KT[j][off:off + 64, k * 128:(k + 1) * 128],
                        QT[j][off:off + 64, q0:q0 + 512],
                        start=True, stop=True)
                es = spool.tile([128, 1024], BF16, tag="es",
                                name=f"es{j}_{qs}_{k}")
                nc.scalar.activation(es[:], pss[:], EXPF, scale=SCALE)
                if prev is not None:
                    emit_pv(k - 1, prev)
                prev = es
            emit_pv(KB - 1, prev)

            # denominators -> reciprocal -> replicate -> scale U rows
            for i, off in ((0, 0), (1, 64)):
                rb = rpool.tile([128, 512], F32, tag="rb",
                                name=f"rb{j}_{qs}_{i}")
                rc = rpool.tile([128, 512], F32, tag="rb",
                                name=f"rc{j}_{qs}_{i}")
                rg = rpool.tile([128, 8], F32, tag="rg",
                                name=f"rg{j}_{qs}_{i}")
                # denom row (512 on one partition) -> spread over 128
                # partitions so the exact reciprocal runs 4 elems/lane
                nc.vector.tensor_copy(rb[64:65, :], psu[i][64:65, :])
                nc.sync.dma_start(rg[:, 0:4], rb[64:65, :])
                nc.vector.reciprocal(rg[:, 4:8], rg[:, 0:4])
                nc.sync.dma_start(rc[0:1, :], rg[:, 4:8])
                nc.gpsimd.partition_broadcast(rc[0:64, :], rc[0:1, :])
                if off == 0:
                    nc.vector.tensor_mul(UT[j][0:64, q0:q0 + 512],
                                         psu[i][0:64, :], rc[0:64, :])
                else:
                    st = stpool.tile([64, 512], F32R, tag="st",
                                     name=f"st{j}_{qs}_{i}")
                    nc.vector.tensor_mul(st[0:64, :], psu[i][0:64, :],
                                         rc[0:64, :])
                    nc.sync.dma_start(UT[j][64:128, q0:q0 + 512],
                                      st[0:64, :])

        def proj_qs(qs):
            for qb in range(qs * 4, qs * 4 + 4):
                plo = psum_u.tile([128, 512], F32, tag="psu", name=f"pl{qb}")
                phi = psum_u.tile([128, 512], F32, tag="psu", name=f"ph{qb}")
                for m in range(MT):
                    lhsT = UT[m][:, qb * 128:(qb + 1) * 128]
                    nc.tensor.matmul(plo[:, 0:512], lhsT, wp_t[m][:, 0:512],
                                     start=(m == 0), stop=(m == MT - 1))
                    nc.tensor.matmul(phi[:, 0:256], lhsT,
                                     wp_t[m][:, 512:768],
                                     start=(m == 0), stop=(m == MT - 1))
                ot = opool.tile([128, C], F32, tag="ost", name=f"ot{qb}")
                nc.vector.tensor_copy(ot[:, 0:512], plo[:, 0:512])
                nc.vector.tensor_copy(ot[:, 512:768], phi[:, 0:256])
                nc.sync.dma_start(out[qb * 128:(qb + 1) * 128, :], ot[:])

        # ---- emission schedule: pipeline QKV m-tiles into attention ----
        VHEAD = min(6, KB)
        qk_mtile(0)
        v_proj(range(VHEAD))
        attn_pair(0, 0, k_hook=lambda k: v_tile(k) if k >= VHEAD else None)
        qk_mtile(1)
        attn_pair(0, 1)
        qk_mtile(2)
        attn_pair(0, 2)
        for qs in range(1, NQ):
            attn_pair(qs, 0)
            proj_qs(qs - 1)
            attn_pair(qs, 1)
            attn_pair(qs, 2)
        proj_qs(NQ - 1)

    nc.compile()
    return nc


_built = {}


def _get_nc(n_tok=2048):
    if n_tok not in _built:
        _built[n_tok] = build(n_tok)
    return _built[n_tok]


def make_in_maps(x, Wqkv, bqkv, Wproj):
    B, NT, _ = x.shape
    x = np.ascontiguousarray(np.asarray(x, dtype=np.float32))
    Wqkv = np.asarray(Wqkv, dtype=np.float32)
    bqkv = np.asarray(bqkv, dtype=np.float32)
    Wproj = np.asarray(Wproj, dtype=np.float32)
    in_maps = []
    for i in range(N_CORES):
        b, g = i // 2, i % 2
        s = g * CHG
        bq = bqkv[s:s + CHG].reshape(MT, 128).T
        bk = bqkv[C + s:C + s + CHG].reshape(MT, 128).T
        in_maps.append({
            "xT": np.ascontiguousarray(x[b].T),
            "wq": np.ascontiguousarray(Wqkv[:, s:s + CHG]),
            "wk": np.ascontiguousarray(Wqkv[:, C + s:C + s + CHG]),
            "wv": np.ascontiguousarray(Wqkv[:, 2 * C + s:2 * C + s + CHG]),
            "wp": np.ascontiguousarray(Wproj[s:s + CHG, :]),
            "bqk": np.ascontiguousarray(
                np.concatenate([bq, bk], axis=1)).astype(np.float32),
            "bv": np.ascontiguousarray(
                bqkv[2 * C + s:2 * C + s + CHG][None, :]).astype(np.float32),
        })
    return in_maps


def gather(results, bproj, B, NT):
    parts = [results[i]["out"] for i in range(N_CORES)]
    out = np.stack([parts[2 * b] + parts[2 * b + 1] for b in range(B)])
    return (out + np.asarray(bproj, np.float32)[None, None, :]).astype(np.float32)


def kernel(x, Wqkv, bqkv, Wproj, bproj, _trace=False):
    x = np.asarray(x)
    B, NT, _ = x.shape
    nc = _get_nc(NT)
    in_maps = make_in_maps(x, Wqkv, bqkv, Wproj)
    res = run_bass_kernel_spmd(nc, in_maps, core_ids=list(range(N_CORES)),
                               trace=_trace)
    out = gather(res.results, bproj, B, NT)
    if _trace:
        return out, res
    return out



# revision 2
# speedup vs baseline: 1.0086x; 1.0086x over previous
"""Multi-head attention block (12 heads, N=2048, C=768) on 8 NeuronCores.

Sharding: core i = (batch b = i//2, head-group g = i%2). Each core computes
attention for 6 heads of one batch plus its slice of the output projection
(row-sharded Wproj); the host sums the two head-group partials per batch.

Per-core dataflow (all matmuls bf16; fp32r runs 2cy/row on HW so bf16
operands ~halve QKV/out-proj tensor time):
  xT [768,2048] bf16 arrives host-transposed; QT/KT [384,2048] bf16 are
  column-major (head h lives at partitions (h%2)*64..+64 of tile h//2), V2
  is token-major with a ones column per head (66th col = pad).

  Heads are processed in pairs (a=2j at PE rows 0-63, b=2j+1 at rows 64-127).
  Per (pair, 512-query chunk qs, key block k):
    S^T_a -> pss[:, 0:512], S^T_b -> pss[:, 512:1024]  (two matmuls in
      disjoint PE row groups sharing one PSUM tile -> concurrent)
    es = exp(S/8) in ONE instruction for both heads: ACT exp for 3 of 4
      k-blocks; for k%4==3 the Vector engine computes a Schraudolph
      bit-trick exp (es_bits = int16(S*A + B), bitcast to bf16; A,B come
      from the `expc` input so they are calibratable without recompile).
      This splits the exp work (the ACT engine is otherwise the 2nd
      bottleneck at ~208us) at a ~0.9% U-error cost.
    U'_a += V2_a[k]^T @ es[:, 0:512], U'_b += V2_b[k]^T @ es[:, 512:1024]
      (PSUM [66,512] accumulated over k; row 64 = softmax denominator via
      the ones column; software-pipelined one k behind the scores)
  U rows are scaled by 1/denominator (DVE reciprocal + gpsimd partition
  broadcast + fused DVE multiply) into UT [384,2048] bf16; odd heads take
  a small DMA hop to land at partitions 64-127.
  out = UT^T-chunks @ Wproj_rows (bf16, PSUM-accumulated), DMA out.
"""

import numpy as np
import ml_dtypes
from contextlib import ExitStack

import concourse.bass as bass
import concourse.tile as tile
from concourse import bacc, mybir
from concourse.bass_utils import run_bass_kernel_spmd

N_CORES = 8
C = 768          # model dim
HG = 6           # heads per core
D = 64           # head dim
CHG = HG * D     # 384, per-group qkv width
CC = C // 128    # 6 contraction chunks
MT = CHG // 128  # 3 m-tiles for QT/KT
SCALE = 1.0 / 8.0

# Schraudolph fast-exp constants (bf16 bitcast):
#   es_bits = int16(S * EXP_A + EXP_B); bits reinterpreted as bf16
# EXP_A = 2^7 * log2(e) * SCALE; EXP_B = 127*2^7 - c with c fitted so the
# multiplicative error vs exp(S/8) has mean 1.0 (so ACT-exact and
# DVE-approx key blocks are mutually unbiased inside one softmax).
EXP_A = 128.0 * np.log2(np.e) * SCALE
EXP_B = 16248.72

F32 = mybir.dt.float32
BF16 = mybir.dt.bfloat16
I16 = mybir.dt.int16

BF = ml_dtypes.bfloat16


def build(n_tok: int = 2048):
    NT = n_tok
    KB = NT // 128           # key blocks
    NQ = NT // 512           # 512-wide query chunks
    EXPF = mybir.ActivationFunctionType.Exp

    nc = bacc.Bacc("TRN2", target_bir_lowering=False, debug=False,
                   num_devices=N_CORES)

    xT = nc.dram_tensor("xT", [C, NT], BF16, kind="ExternalInput").ap()
    wq = nc.dram_tensor("wq", [C, CHG], BF16, kind="ExternalInput").ap()
    wk = nc.dram_tensor("wk", [C, CHG], BF16, kind="ExternalInput").ap()
    wv = nc.dram_tensor("wv", [C, CHG], BF16, kind="ExternalInput").ap()
    wp = nc.dram_tensor("wp", [CHG, C], BF16, kind="ExternalInput").ap()
    bqk = nc.dram_tensor("bqk", [128, 2 * MT], F32, kind="ExternalInput").ap()
    bv = nc.dram_tensor("bv", [1, CHG], F32, kind="ExternalInput").ap()
    expc = nc.dram_tensor("expc", [128, 2], F32, kind="ExternalInput").ap()
    out = nc.dram_tensor("out", [NT, C], F32, kind="ExternalOutput").ap()

    with tile.TileContext(nc) as tc, ExitStack() as ctx:
        wpool = ctx.enter_context(tc.tile_pool(name="w", bufs=1))
        perm = ctx.enter_context(tc.tile_pool(name="perm", bufs=1))
        psum = ctx.enter_context(tc.tile_pool(name="ps", bufs=2, space="PSUM"))
        psum_u = ctx.enter_context(tc.tile_pool(name="psu", bufs=4,
                                                space="PSUM"))

        # ---- persistent SBUF ----
        wq_t = [wpool.tile([128, CHG], BF16, tag=f"wq{c}", name=f"wq{c}")
                for c in range(CC)]
        wk_t = [wpool.tile([128, CHG], BF16, tag=f"wk{c}", name=f"wk{c}")
                for c in range(CC)]
        wv_t = [wpool.tile([128, CHG], BF16, tag=f"wv{c}", name=f"wv{c}")
                for c in range(CC)]
        wp_t = [wpool.tile([128, C], BF16, tag=f"wp{m}", name=f"wp{m}")
                for m in range(MT)]
        bqk_t = wpool.tile([128, 2 * MT], F32, tag="bqk")
        bv_row = wpool.tile([1, CHG], F32, tag="bvr")
        bv_bc = wpool.tile([128, CHG], F32, tag="bvb")
        expc_t = wpool.tile([128, 2], F32, tag="expc")

        QT = [perm.tile([128, NT], BF16, tag=f"qt{m}", name=f"qtt{m}")
              for m in range(MT)]
        KT = [perm.tile([128, NT], BF16, tag=f"kt{m}", name=f"ktt{m}")
              for m in range(MT)]
        V2 = [perm.tile([128, HG, 66], BF16, tag=f"v2{t}", name=f"v2t{t}")
              for t in range(KB)]
        UT = [perm.tile([128, NT], BF16, tag=f"ut{m}", name=f"utt{m}")
              for m in range(MT)]

        # ---- input DMA (ordered so qk_mtile(0) can start early) ----
        nc.sync.dma_start(bqk_t[:], bqk)
        nc.sync.dma_start(expc_t[:], expc)
        for c in range(CC):
            nc.sync.dma_start(wq_t[c][:], wq[c * 128:(c + 1) * 128, :])
            nc.sync.dma_start(wk_t[c][:], wk[c * 128:(c + 1) * 128, :])

        spool = ctx.enter_context(tc.tile_pool(name="es", bufs=14))
        rpool = ctx.enter_context(tc.tile_pool(name="rb", bufs=4))
        stpool = ctx.enter_context(tc.tile_pool(name="st", bufs=3))
        opool = ctx.enter_context(tc.tile_pool(name="ost", bufs=3))
        xpool = ctx.enter_context(tc.tile_pool(name="xt", bufs=1))

        # ---- QKV projection pieces ----
        xt = []
        for c in range(CC):
            xc = xpool.tile([128, NT], BF16, tag=f"x{c}", name=f"xt{c}")
            nc.sync.dma_start(xc[:], xT[c * 128:(c + 1) * 128, :])
            xt.append(xc)

        for c in range(CC):
            nc.sync.dma_start(wv_t[c][:], wv[c * 128:(c + 1) * 128, :])
        nc.sync.dma_start(bv_row[0:1, :], bv[0:1, :])
        for m in range(MT):
            nc.sync.dma_start(wp_t[m][:], wp[m * 128:(m + 1) * 128, :])
        nc.gpsimd.partition_broadcast(bv_bc[:], bv_row[0:1, :])
        for t in range(KB):
            nc.vector.tensor_scalar(
                V2[t][:, :, 64:66],
                bv_bc[:, 0:12].rearrange("p (a b) -> p a b", a=HG),
                0.0, 1.0, mybir.AluOpType.mult, mybir.AluOpType.add)

        def qk_mtile(m):
            for wt, dst, bcol in ((wq_t, QT, m), (wk_t, KT, MT + m)):
                for n in range(NQ):
                    ps = psum.tile([128, 512], F32, tag="ps",
                                   name=f"psqk{m}_{n}")
                    for c in range(CC):
                        nc.tensor.matmul(
                            ps[:], wt[c][:, m * 128:(m + 1) * 128],
                            xt[c][:, n * 512:(n + 1) * 512],
                            start=(c == 0), stop=(c == CC - 1))
                    nc.vector.tensor_scalar_add(
                        dst[m][:, n * 512:(n + 1) * 512], ps[:],
                        bqk_t[:, bcol:bcol + 1])

        def v_tile(t):
            ps = psum_u.tile([128, CHG], F32, tag="psu", name=f"psv{t}")
            for c in range(CC):
                nc.tensor.matmul(ps[:], xt[c][:, t * 128:(t + 1) * 128],
                                 wv_t[c][:],
                                 start=(c == 0), stop=(c == CC - 1))
            nc.vector.tensor_add(
                V2[t][:, :, 0:64],
                ps[:].rearrange("p (h d) -> p h d", h=HG),
                bv_bc[:].rearrange("p (h d) -> p h d", h=HG))

        def v_proj(ts):
            for t in ts:
                v_tile(t)

        # ---- attention pieces ----
        def attn_pair(qs, j, k_hook=None):
            ha, hb = 2 * j, 2 * j + 1
            q0 = qs * 512
            psu = [psum_u.tile([128, 512], F32, tag="psu",
                               name=f"psu{j}_{qs}_{i}") for i in range(2)]

            def emit_pv(k, es):
                for i, h in enumerate((ha, hb)):
                    nc.tensor.matmul(
                        psu[i][0:66, :], V2[k][:, h, :],
                        es[:, i * 512:(i + 1) * 512],
                        start=(k == 0), stop=(k == KB - 1))

            prev = None
            for k in range(KB):
                if k_hook is not None:
                    k_hook(k)
                pss = psum.tile([128, 1024], F32, tag="ps",
                                name=f"pss{j}_{qs}_{k}")
                # head a in PE rows 0-63 -> pss cols 0-511, head b in
                # rows 64-127 -> cols 512-1023; one shared PSUM tile so
                # the second matmul has no semaphore wait and the pair
                # runs concurrently in disjoint row groups.
                for i, off in ((0, 0), (1, 64)):
                    nc.tensor.matmul(
                        pss[:, i * 512:(i + 1) * 512],
                        KT[j][off:off + 64, k * 128:(k + 1) * 128],
                        QT[j][off:off + 64, q0:q0 + 512],
                        start=True, stop=True)
                es = spool.tile([128, 1024], BF16, tag="es",
                                name=f"es{j}_{qs}_{k}")
                if k % 4 == 3:
                    # DVE fast exp: bits = int16(S*A + B) -> bf16
                    nc.vector.tensor_scalar(
                        es[:].bitcast(I16), pss[:],
                        expc_t[:, 0:1], expc_t[:, 1:2],
                        mybir.AluOpType.mult, mybir.AluOpType.add)
                else:
                    nc.scalar.activation(es[:], pss[:], EXPF, scale=SCALE)
                if prev is not None:
                    emit_pv(k - 1, prev)
                prev = es
            emit_pv(KB - 1, prev)

            # denominators -> reciprocal -> replicate -> scale U rows
            for i, off in ((0, 0), (1, 64)):
                rb = rpool.tile([128, 512], F32, tag="rb",
                                name=f"rb{j}_{qs}_{i}")
                rc = rpool.tile([128, 512], F32, tag="rb",
                                name=f"rc{j}_{qs}_{i}")
                rg = rpool.tile([128, 8], F32, tag="rg",
                                name=f"rg{j}_{qs}_{i}")
                # denom row (512 on one partition) -> spread over 128
                # partitions so the exact reciprocal runs 4 elems/lane
                nc.vector.tensor_copy(rb[64:65, :], psu[i][64:65, :])
                nc.sync.dma_start(rg[:, 0:4], rb[64:65, :])
                nc.vector.reciprocal(rg[:, 4:8], rg[:, 0:4])
                nc.sync.dma_start(rc[0:1, :], rg[:, 4:8])
                nc.gpsimd.partition_broadcast(rc[0:64, :], rc[0:1, :])
                if off == 0:
                    nc.vector.tensor_mul(UT[j][0:64, q0:q0 + 512],
                                         psu[i][0:64, :], rc[0:64, :])
                else:
                    st = stpool.tile([64, 512], BF16, tag="st",
                                     name=f"st{j}_{qs}_{i}")
                    nc.vector.tensor_mul(st[0:64, :], psu[i][0:64, :],
                                         rc[0:64, :])
                    nc.sync.dma_start(UT[j][64:128, q0:q0 + 512],
                                      st[0:64, :])

        def proj_qs(qs):
            for qb in range(qs * 4, qs * 4 + 4):
                plo = psum_u.tile([128, 512], F32, tag="psu", name=f"pl{qb}")
                phi = psum_u.tile([128, 512], F32, tag="psu", name=f"ph{qb}")
                for m in range(MT):
                    lhsT = UT[m][:, qb * 128:(qb + 1) * 128]
                    nc.tensor.matmul(plo[:, 0:512], lhsT, wp_t[m][:, 0:512],
                                     start=(m == 0), stop=(m == MT - 1))
                    nc.tensor.matmul(phi[:, 0:256], lhsT,
                                     wp_t[m][:, 512:768],
                                     start=(m == 0), stop=(m == MT - 1))
                ot = opool.tile([128, C], F32, tag="ost", name=f"ot{qb}")
                nc.vector.tensor_copy(ot[:, 0:512], plo[:, 0:512])
                nc.vector.tensor_copy(ot[:, 512:768], phi[:, 0:256])
                nc.sync.dma_start(out[qb * 128:(qb + 1) * 128, :], ot[:])

        # ---- emission schedule: pipeline QKV m-tiles into attention ----
        VHEAD = min(6, KB)
        qk_mtile(0)
        v_proj(range(VHEAD))
        attn_pair(0, 0, k_hook=lambda k: v_tile(k) if k >= VHEAD else None)
        qk_mtile(1)
        attn_pair(0, 1)
        qk_mtile(2)
        attn_pair(0, 2)
        for qs in range(1, NQ):
            attn_pair(qs, 0)
            proj_qs(qs - 1)
            attn_pair(qs, 1)
            attn_pair(qs, 2)
        proj_qs(NQ - 1)

    nc.compile()
    return nc


_built = {}


def _get_nc(n_tok=2048):
    if n_tok not in _built:
        _built[n_tok] = build(n_tok)
    return _built[n_tok]


def make_in_maps(x, Wqkv, bqkv, Wproj, exp_b=EXP_B):
    B, NT, _ = x.shape
    x = np.ascontiguousarray(np.asarray(x, dtype=np.float32))
    Wqkv = np.asarray(Wqkv, dtype=np.float32)
    bqkv = np.asarray(bqkv, dtype=np.float32)
    Wproj = np.asarray(Wproj, dtype=np.float32)
    expc = np.zeros((128, 2), dtype=np.float32)
    expc[:, 0] = EXP_A
    expc[:, 1] = exp_b
    in_maps = []
    for i in range(N_CORES):
        b, g = i // 2, i % 2
        s = g * CHG
        bq = bqkv[s:s + CHG].reshape(MT, 128).T
        bk = bqkv[C + s:C + s + CHG].reshape(MT, 128).T
        in_maps.append({
            "xT": np.ascontiguousarray(x[b].T.astype(BF)),
            "wq": np.ascontiguousarray(Wqkv[:, s:s + CHG].astype(BF)),
            "wk": np.ascontiguousarray(Wqkv[:, C + s:C + s + CHG].astype(BF)),
            "wv": np.ascontiguousarray(
                Wqkv[:, 2 * C + s:2 * C + s + CHG].astype(BF)),
            "wp": np.ascontiguousarray(Wproj[s:s + CHG, :].astype(BF)),
            "bqk": np.ascontiguousarray(
                np.concatenate([bq, bk], axis=1)).astype(np.float32),
            "bv": np.ascontiguousarray(
                bqkv[2 * C + s:2 * C + s + CHG][None, :]).astype(np.float32),
            "expc": expc,
        })
    return in_maps


def gather(results, bproj, B, NT):
    parts = [results[i]["out"] for i in range(N_CORES)]
    out = np.stack([parts[2 * b] + parts[2 * b + 1] for b in range(B)])
    return (out + np.asarray(bproj, np.float32)[None, None, :]).astype(np.float32)


def kernel(x, Wqkv, bqkv, Wproj, bproj, _trace=False, _exp_b=EXP_B):
    x = np.asarray(x)
    B, NT, _ = x.shape
    nc = _get_nc(NT)
    in_maps = make_in_maps(x, Wqkv, bqkv, Wproj, exp_b=_exp_b)
    res = run_bass_kernel_spmd(nc, in_maps, core_ids=list(range(N_CORES)),
                               trace=_trace)
    out = gather(res.results, bproj, B, NT)
    if _trace:
        return out, res
    return out


# revision 7
# speedup vs baseline: 1.0272x; 1.0185x over previous
"""Multi-head attention block (12 heads, N=2048, C=768) on 8 NeuronCores.

Sharding: core i = (batch b = i//2, head-group g = i%2). Each core computes
attention for 6 heads of one batch plus its slice of the output projection
(row-sharded Wproj); the host sums the two head-group partials per batch.

Per-core dataflow (all matmuls bf16; fp32r runs 2cy/row on HW so bf16
operands ~halve QKV/out-proj tensor time):
  xT [768,2048] bf16 arrives host-transposed; QT/KT [384,2048] bf16 are
  column-major (head h lives at partitions (h%2)*64..+64 of tile h//2), V2
  is token-major with a ones column per head (66th col = pad).

  Heads are processed in pairs (a=2j at PE rows 0-63, b=2j+1 at rows 64-127).
  Per (pair, 512-query chunk qs, key block k):
    S^T_a -> pss[:, 0:512], S^T_b -> pss[:, 512:1024]  (two matmuls in
      disjoint PE row groups sharing one PSUM tile -> concurrent)
    es = exp(S/8) in ONE instruction for both heads: ACT exp for 3 of 4
      k-blocks; for k%4==3 the Vector engine computes a Schraudolph
      bit-trick exp (es_bits = int16(S*A + B), bitcast to bf16; A,B come
      from the `expc` input so they are calibratable without recompile).
      This splits the exp work (the ACT engine is otherwise the 2nd
      bottleneck at ~208us) at a ~0.9% U-error cost.
    U'_a += V2_a[k]^T @ es[:, 0:512], U'_b += V2_b[k]^T @ es[:, 512:1024]
      (PSUM [66,512] accumulated over k; row 64 = softmax denominator via
      the ones column; software-pipelined one k behind the scores)
  U rows are scaled by 1/denominator (DVE reciprocal + gpsimd partition
  broadcast + fused DVE multiply) into UT [384,2048] bf16; odd heads take
  a small DMA hop to land at partitions 64-127.
  out = UT^T-chunks @ Wproj_rows (bf16, PSUM-accumulated), DMA out.
"""

import numpy as np
import ml_dtypes
from contextlib import ExitStack

import concourse.bass as bass
import concourse.tile as tile
from concourse import bacc, mybir
from concourse.bass_utils import run_bass_kernel_spmd

N_CORES = 8
C = 768          # model dim
HG = 6           # heads per core
D = 64           # head dim
CHG = HG * D     # 384, per-group qkv width
CC = C // 128    # 6 contraction chunks
MT = CHG // 128  # 3 m-tiles for QT/KT
SCALE = 1.0 / 8.0

# Schraudolph fast-exp constants (bf16 bitcast):
#   es_bits = int16(S * EXP_A + EXP_B); bits reinterpreted as bf16
# EXP_A = 2^7 * log2(e) * SCALE; EXP_B = 127*2^7 - c with c fitted so the
# multiplicative error vs exp(S/8) has mean 1.0 (so ACT-exact and
# DVE-approx key blocks are mutually unbiased inside one softmax).
EXP_A = 128.0 * np.log2(np.e) * SCALE
EXP_B = 16248.72

F32 = mybir.dt.float32
BF16 = mybir.dt.bfloat16
I16 = mybir.dt.int16

BF = ml_dtypes.bfloat16


def build(n_tok: int = 2048):
    NT = n_tok
    KB = NT // 128           # key blocks
    NQ = NT // 512           # 512-wide query chunks
    EXPF = mybir.ActivationFunctionType.Exp

    nc = bacc.Bacc("TRN2", target_bir_lowering=False, debug=False,
                   num_devices=N_CORES)

    xT = nc.dram_tensor("xT", [C, NT], BF16, kind="ExternalInput").ap()
    wq = nc.dram_tensor("wq", [C, CHG], BF16, kind="ExternalInput").ap()
    wk = nc.dram_tensor("wk", [C, CHG], BF16, kind="ExternalInput").ap()
    wv = nc.dram_tensor("wv", [C, CHG], BF16, kind="ExternalInput").ap()
    wp = nc.dram_tensor("wp", [CHG, C], BF16, kind="ExternalInput").ap()
    bqk = nc.dram_tensor("bqk", [128, 2 * MT], F32, kind="ExternalInput").ap()
    bv = nc.dram_tensor("bv", [1, CHG], F32, kind="ExternalInput").ap()
    expc = nc.dram_tensor("expc", [128, 2], F32, kind="ExternalInput").ap()
    out = nc.dram_tensor("out", [NT, C], BF16, kind="ExternalOutput").ap()

    with tile.TileContext(nc) as tc, ExitStack() as ctx:
        wpool = ctx.enter_context(tc.tile_pool(name="w", bufs=1))
        perm = ctx.enter_context(tc.tile_pool(name="perm", bufs=1))
        psum = ctx.enter_context(tc.tile_pool(name="ps", bufs=2, space="PSUM"))
        psum_u = ctx.enter_context(tc.tile_pool(name="psu", bufs=4,
                                                space="PSUM"))

        # ---- persistent SBUF ----
        wq_t = [wpool.tile([128, CHG], BF16, tag=f"wq{c}", name=f"wq{c}")
                for c in range(CC)]
        wk_t = [wpool.tile([128, CHG], BF16, tag=f"wk{c}", name=f"wk{c}")
                for c in range(CC)]
        wv_t = [wpool.tile([128, CHG], BF16, tag=f"wv{c}", name=f"wv{c}")
                for c in range(CC)]
        wp_t = [wpool.tile([128, C], BF16, tag=f"wp{m}", name=f"wp{m}")
                for m in range(MT)]
        bqk_t = wpool.tile([128, 2 * MT], F32, tag="bqk")
        bv_row = wpool.tile([1, CHG], F32, tag="bvr")
        bv_bc = wpool.tile([128, CHG], F32, tag="bvb")
        expc_t = wpool.tile([128, 2], F32, tag="expc")

        QT = [perm.tile([128, NT], BF16, tag=f"qt{m}", name=f"qtt{m}")
              for m in range(MT)]
        KT = [perm.tile([128, NT], BF16, tag=f"kt{m}", name=f"ktt{m}")
              for m in range(MT)]
        V2 = [perm.tile([128, HG, 66], BF16, tag=f"v2{t}", name=f"v2t{t}")
              for t in range(KB)]
        UT = [perm.tile([128, NT], BF16, tag=f"ut{m}", name=f"utt{m}")
              for m in range(MT)]

        # ---- input DMA (interleaved per contraction chunk so qk_mtile(0)
        # matmul c unblocks as soon as wq[c]/wk[c]/xt[c] land) ----
        nc.sync.dma_start(bqk_t[:], bqk)
        nc.sync.dma_start(expc_t[:], expc)

        spool = ctx.enter_context(tc.tile_pool(name="es", bufs=14))
        rpool = ctx.enter_context(tc.tile_pool(name="rb", bufs=4))
        stpool = ctx.enter_context(tc.tile_pool(name="st", bufs=3))
        opool = ctx.enter_context(tc.tile_pool(name="ost", bufs=3))
        xpool = ctx.enter_context(tc.tile_pool(name="xt", bufs=1))

        # ---- QKV projection pieces ----
        xt = [xpool.tile([128, NT], BF16, tag=f"x{c}", name=f"xt{c}")
              for c in range(CC)]
        for c in range(CC):
            nc.sync.dma_start(wq_t[c][:], wq[c * 128:(c + 1) * 128, :])
            nc.sync.dma_start(wk_t[c][:], wk[c * 128:(c + 1) * 128, :])
            nc.sync.dma_start(xt[c][:], xT[c * 128:(c + 1) * 128, :])

        for c in range(CC):
            nc.sync.dma_start(wv_t[c][:], wv[c * 128:(c + 1) * 128, :])
        nc.sync.dma_start(bv_row[0:1, :], bv[0:1, :])
        for m in range(MT):
            nc.sync.dma_start(wp_t[m][:], wp[m * 128:(m + 1) * 128, :])
        nc.gpsimd.partition_broadcast(bv_bc[:], bv_row[0:1, :])
        for t in range(KB):
            nc.gpsimd.tensor_scalar(
                V2[t][:, :, 64:66],
                bv_bc[:, 0:12].rearrange("p (a b) -> p a b", a=HG),
                0.0, 1.0, mybir.AluOpType.mult, mybir.AluOpType.add)

        def qk_mtile(m):
            for wt, dst, bcol in ((wq_t, QT, m), (wk_t, KT, MT + m)):
                for n in range(NQ):
                    ps = psum.tile([128, 512], F32, tag="ps",
                                   name=f"psqk{m}_{n}")
                    for c in range(CC):
                        nc.tensor.matmul(
                            ps[:], wt[c][:, m * 128:(m + 1) * 128],
                            xt[c][:, n * 512:(n + 1) * 512],
                            start=(c == 0), stop=(c == CC - 1))
                    nc.vector.tensor_scalar_add(
                        dst[m][:, n * 512:(n + 1) * 512], ps[:],
                        bqk_t[:, bcol:bcol + 1])

        def v_tile(t):
            ps = psum_u.tile([128, CHG], F32, tag="psu", name=f"psv{t}")
            for c in range(CC):
                nc.tensor.matmul(ps[:], xt[c][:, t * 128:(t + 1) * 128],
                                 wv_t[c][:],
                                 start=(c == 0), stop=(c == CC - 1))
            nc.vector.tensor_add(
                V2[t][:, :, 0:64],
                ps[:].rearrange("p (h d) -> p h d", h=HG),
                bv_bc[:].rearrange("p (h d) -> p h d", h=HG))

        def v_proj(ts):
            for t in ts:
                v_tile(t)

        # ---- attention pieces ----
        def attn_pair(qs, j, k_hook=None):
            ha, hb = 2 * j, 2 * j + 1
            q0 = qs * 512
            psu = [psum_u.tile([128, 512], F32, tag="psu",
                               name=f"psu{j}_{qs}_{i}") for i in range(2)]

            def emit_pv(k, es):
                for i, h in enumerate((ha, hb)):
                    nc.tensor.matmul(
                        psu[i][0:66, :], V2[k][:, h, :],
                        es[:, i * 512:(i + 1) * 512],
                        start=(k == 0), stop=(k == KB - 1))

            prev = None
            for k in range(KB):
                if k_hook is not None:
                    k_hook(k)
                pss = psum.tile([128, 1024], F32, tag="ps",
                                name=f"pss{j}_{qs}_{k}")
                # head a in PE rows 0-63 -> pss cols 0-511, head b in
                # rows 64-127 -> cols 512-1023; one shared PSUM tile so
                # the second matmul has no semaphore wait and the pair
                # runs concurrently in disjoint row groups.
                for i, off in ((0, 0), (1, 64)):
                    nc.tensor.matmul(
                        pss[:, i * 512:(i + 1) * 512],
                        KT[j][off:off + 64, k * 128:(k + 1) * 128],
                        QT[j][off:off + 64, q0:q0 + 512],
                        start=True, stop=True)
                es = spool.tile([128, 1024], BF16, tag="es",
                                name=f"es{j}_{qs}_{k}")
                if k % 16 in (1, 3, 5, 8, 10, 12, 14):
                    # DVE fast exp: bits = int16(S*A + B) -> bf16
                    nc.vector.tensor_scalar(
                        es[:].bitcast(I16), pss[:],
                        expc_t[:, 0:1], expc_t[:, 1:2],
                        mybir.AluOpType.mult, mybir.AluOpType.add)
                else:
                    nc.scalar.activation(es[:], pss[:], EXPF, scale=SCALE)
                if prev is not None:
                    emit_pv(k - 1, prev)
                prev = es
            emit_pv(KB - 1, prev)

            # denominators -> reciprocal -> replicate -> scale U rows
            for i, off in ((0, 0), (1, 64)):
                rb = rpool.tile([128, 512], F32, tag="rb",
                                name=f"rb{j}_{qs}_{i}")
                rc = rpool.tile([128, 512], F32, tag="rb",
                                name=f"rc{j}_{qs}_{i}")
                rg = rpool.tile([128, 8], F32, tag="rg",
                                name=f"rg{j}_{qs}_{i}")
                # denom row (512 on one partition) -> spread over 128
                # partitions so the exact reciprocal runs 4 elems/lane
                nc.vector.tensor_copy(rb[64:65, :], psu[i][64:65, :])
                nc.sync.dma_start(rg[:, 0:4], rb[64:65, :])
                nc.vector.reciprocal(rg[:, 4:8], rg[:, 0:4])
                nc.sync.dma_start(rc[0:1, :], rg[:, 4:8])
                nc.gpsimd.partition_broadcast(rc[0:64, :], rc[0:1, :])
                if off == 0:
                    nc.vector.tensor_mul(UT[j][0:64, q0:q0 + 512],
                                         psu[i][0:64, :], rc[0:64, :])
                else:
                    st = stpool.tile([64, 512], BF16, tag="st",
                                     name=f"st{j}_{qs}_{i}")
                    nc.vector.tensor_mul(st[0:64, :], psu[i][0:64, :],
                                         rc[0:64, :])
                    nc.sync.dma_start(UT[j][64:128, q0:q0 + 512],
                                      st[0:64, :])

        def proj_qs(qs):
            for qb in range(qs * 4, qs * 4 + 4):
                plo = psum_u.tile([128, 512], F32, tag="psu", name=f"pl{qb}")
                phi = psum_u.tile([128, 512], F32, tag="psu", name=f"ph{qb}")
                for m in range(MT):
                    lhsT = UT[m][:, qb * 128:(qb + 1) * 128]
                    nc.tensor.matmul(plo[:, 0:512], lhsT, wp_t[m][:, 0:512],
                                     start=(m == 0), stop=(m == MT - 1))
                    nc.tensor.matmul(phi[:, 0:256], lhsT,
                                     wp_t[m][:, 512:768],
                                     start=(m == 0), stop=(m == MT - 1))
                ot = opool.tile([128, C], BF16, tag="ost", name=f"ot{qb}")
                nc.scalar.copy(ot[:, 0:512], plo[:, 0:512])
                nc.scalar.copy(ot[:, 512:768], phi[:, 0:256])
                nc.sync.dma_start(out[qb * 128:(qb + 1) * 128, :], ot[:])

        # ---- emission schedule: pipeline QKV m-tiles into attention ----
        VHEAD = min(6, KB)
        qk_mtile(0)
        v_proj(range(VHEAD))
        attn_pair(0, 0, k_hook=lambda k: v_tile(k) if k >= VHEAD else None)
        qk_mtile(1)
        attn_pair(0, 1)
        qk_mtile(2)
        attn_pair(0, 2)
        for qs in range(1, NQ):
            attn_pair(qs, 0)
            proj_qs(qs - 1)
            attn_pair(qs, 1)
            attn_pair(qs, 2)
        proj_qs(NQ - 1)

    nc.compile()
    return nc


_built = {}


def _get_nc(n_tok=2048):
    if n_tok not in _built:
        _built[n_tok] = build(n_tok)
    return _built[n_tok]


def make_in_maps(x, Wqkv, bqkv, Wproj, exp_b=EXP_B):
    B, NT, _ = x.shape
    x = np.ascontiguousarray(np.asarray(x, dtype=np.float32))
    Wqkv = np.asarray(Wqkv, dtype=np.float32)
    bqkv = np.asarray(bqkv, dtype=np.float32)
    Wproj = np.asarray(Wproj, dtype=np.float32)
    expc = np.zeros((128, 2), dtype=np.float32)
    expc[:, 0] = EXP_A
    expc[:, 1] = exp_b
    in_maps = []
    for i in range(N_CORES):
        b, g = i // 2, i % 2
        s = g * CHG
        bq = bqkv[s:s + CHG].reshape(MT, 128).T
        bk = bqkv[C + s:C + s + CHG].reshape(MT, 128).T
        in_maps.append({
            "xT": np.ascontiguousarray(x[b].T.astype(BF)),
            "wq": np.ascontiguousarray(Wqkv[:, s:s + CHG].astype(BF)),
            "wk": np.ascontiguousarray(Wqkv[:, C + s:C + s + CHG].astype(BF)),
            "wv": np.ascontiguousarray(
                Wqkv[:, 2 * C + s:2 * C + s + CHG].astype(BF)),
            "wp": np.ascontiguousarray(Wproj[s:s + CHG, :].astype(BF)),
            "bqk": np.ascontiguousarray(
                np.concatenate([bq, bk], axis=1)).astype(np.float32),
            "bv": np.ascontiguousarray(
                bqkv[2 * C + s:2 * C + s + CHG][None, :]).astype(np.float32),
            "expc": expc,
        })
    return in_maps


def gather(results, bproj, B, NT):
    parts = [np.asarray(results[i]["out"], dtype=np.float32)
             for i in range(N_CORES)]
    out = np.stack([parts[2 * b] + parts[2 * b + 1] for b in range(B)])
    return (out + np.asarray(bproj, np.float32)[None, None, :]).astype(np.float32)


def kernel(x, Wqkv, bqkv, Wproj, bproj, _trace=False, _exp_b=EXP_B):
    x = np.asarray(x)
    B, NT, _ = x.shape
    nc = _get_nc(NT)
    in_maps = make_in_maps(x, Wqkv, bqkv, Wproj, exp_b=_exp_b)
    res = run_bass_kernel_spmd(nc, in_maps, core_ids=list(range(N_CORES)),
                               trace=_trace)
    out = gather(res.results, bproj, B, NT)
    if _trace:
        return out, res
    return out


# revision 8
# speedup vs baseline: 1.1205x; 1.0908x over previous
"""Multi-head attention block (12 heads, N=2048, C=768) on 8 NeuronCores.

Sharding: core i = (batch b = i//2, head-group g = i%2). Each core computes
attention for 6 heads of one batch plus its slice of the output projection
(row-sharded Wproj); the host sums the two head-group partials per batch.

Per-core dataflow (all matmuls bf16; fp32r runs 2cy/row on HW so bf16
operands ~halve QKV/out-proj tensor time):
  xT [768,2048] bf16 arrives host-transposed; QT/KT [384,2048] bf16 are
  column-major (head h lives at partitions (h%2)*64..+64 of tile h//2), V2
  is token-major with a ones column per head (66th col = pad).

  Heads are processed in pairs (a=2j at PE rows 0-63, b=2j+1 at rows 64-127).
  Per (pair, 512-query chunk qs, key block k):
    S^T_a -> pss[:, 0:512], S^T_b -> pss[:, 512:1024]  (two matmuls in
      disjoint PE row groups sharing one PSUM tile -> concurrent)
    es = exp(S/8) in ONE instruction for both heads: ACT exp for 3 of 4
      k-blocks; for k%4==3 the Vector engine computes a Schraudolph
      bit-trick exp (es_bits = int16(S*A + B), bitcast to bf16; A,B come
      from the `expc` input so they are calibratable without recompile).
      This splits the exp work (the ACT engine is otherwise the 2nd
      bottleneck at ~208us) at a ~0.9% U-error cost.
    U'_a += V2_a[k]^T @ es[:, 0:512], U'_b += V2_b[k]^T @ es[:, 512:1024]
      (PSUM [66,512] accumulated over k; row 64 = softmax denominator via
      the ones column; software-pipelined one k behind the scores)
  U rows are scaled by 1/denominator (DVE reciprocal + gpsimd partition
  broadcast + fused DVE multiply) into UT [384,2048] bf16; odd heads take
  a small DMA hop to land at partitions 64-127.
  out = UT^T-chunks @ Wproj_rows (bf16, PSUM-accumulated), DMA out.
"""

import numpy as np
import ml_dtypes
from contextlib import ExitStack

import concourse.bass as bass
import concourse.tile as tile
from concourse import bacc, mybir
from concourse.bass_utils import run_bass_kernel_spmd

N_CORES = 8
C = 768          # model dim
HG = 6           # heads per core
D = 64           # head dim
CHG = HG * D     # 384, per-group qkv width
CC = C // 128    # 6 contraction chunks
MT = CHG // 128  # 3 m-tiles for QT/KT
SCALE = 1.0 / 8.0

# Schraudolph fast-exp constants (bf16 bitcast):
#   es_bits = int16(S * EXP_A + EXP_B); bits reinterpreted as bf16
# EXP_A = 2^7 * log2(e) * SCALE; EXP_B = 127*2^7 - c with c fitted so the
# multiplicative error vs exp(S/8) has mean 1.0 (so ACT-exact and
# DVE-approx key blocks are mutually unbiased inside one softmax).
EXP_A = 128.0 * np.log2(np.e) * SCALE
EXP_B = 16248.72

F32 = mybir.dt.float32
BF16 = mybir.dt.bfloat16
I16 = mybir.dt.int16

BF = ml_dtypes.bfloat16


def build(n_tok: int = 2048):
    NT = n_tok
    KB = NT // 128           # key blocks
    NQ = NT // 512           # 512-wide query chunks
    EXPF = mybir.ActivationFunctionType.Exp

    nc = bacc.Bacc("TRN2", target_bir_lowering=False, debug=False,
                   num_devices=N_CORES)

    xT = nc.dram_tensor("xT", [C, NT], BF16, kind="ExternalInput").ap()
    wq = nc.dram_tensor("wq", [C, CHG], BF16, kind="ExternalInput").ap()
    wk = nc.dram_tensor("wk", [C, CHG], BF16, kind="ExternalInput").ap()
    wv = nc.dram_tensor("wv", [C, CHG], BF16, kind="ExternalInput").ap()
    wp = nc.dram_tensor("wp", [CHG, C], BF16, kind="ExternalInput").ap()
    bqk = nc.dram_tensor("bqk", [128, 2 * MT], F32, kind="ExternalInput").ap()
    bv = nc.dram_tensor("bv", [1, CHG], F32, kind="ExternalInput").ap()
    expc = nc.dram_tensor("expc", [128, 2], F32, kind="ExternalInput").ap()
    out = nc.dram_tensor("out", [NT, C], BF16, kind="ExternalOutput").ap()

    with tile.TileContext(nc) as tc, ExitStack() as ctx:
        wpool = ctx.enter_context(tc.tile_pool(name="w", bufs=1))
        perm = ctx.enter_context(tc.tile_pool(name="perm", bufs=1))
        psum = ctx.enter_context(tc.tile_pool(name="ps", bufs=2, space="PSUM"))
        psum_u = ctx.enter_context(tc.tile_pool(name="psu", bufs=4,
                                                space="PSUM"))

        # ---- persistent SBUF ----
        wq_t = [wpool.tile([128, CHG], BF16, tag=f"wq{c}", name=f"wq{c}")
                for c in range(CC)]
        wk_t = [wpool.tile([128, CHG], BF16, tag=f"wk{c}", name=f"wk{c}")
                for c in range(CC)]
        wv_t = [wpool.tile([128, CHG], BF16, tag=f"wv{c}", name=f"wv{c}")
                for c in range(CC)]
        wp_t = [wpool.tile([128, C], BF16, tag=f"wp{m}", name=f"wp{m}")
                for m in range(MT)]
        bqk_t = wpool.tile([128, 2 * MT], F32, tag="bqk")
        bv_row = wpool.tile([1, CHG], F32, tag="bvr")
        bv_bc = wpool.tile([128, CHG], F32, tag="bvb")
        expc_t = wpool.tile([128, 2], F32, tag="expc")

        QT = [perm.tile([128, NT], BF16, tag=f"qt{m}", name=f"qtt{m}")
              for m in range(MT)]
        KT = [perm.tile([128, NT], BF16, tag=f"kt{m}", name=f"ktt{m}")
              for m in range(MT)]
        V2 = [perm.tile([128, HG, 66], BF16, tag=f"v2{t}", name=f"v2t{t}")
              for t in range(KB)]
        UT = [perm.tile([128, NT], BF16, tag=f"ut{m}", name=f"utt{m}")
              for m in range(MT)]

        # ---- input DMA (interleaved per contraction chunk so qk_mtile(0)
        # matmul c unblocks as soon as wq[c]/wk[c]/xt[c] land) ----
        nc.sync.dma_start(bqk_t[:], bqk)
        nc.sync.dma_start(expc_t[:], expc)

        spool = ctx.enter_context(tc.tile_pool(name="es", bufs=14))
        rpool = ctx.enter_context(tc.tile_pool(name="rb", bufs=4))
        stpool = ctx.enter_context(tc.tile_pool(name="st", bufs=3))
        opool = ctx.enter_context(tc.tile_pool(name="ost", bufs=3))
        xpool = ctx.enter_context(tc.tile_pool(name="xt", bufs=1))

        # ---- QKV projection pieces ----
        xt = [xpool.tile([128, NT], BF16, tag=f"x{c}", name=f"xt{c}")
              for c in range(CC)]
        for c in range(CC):
            nc.sync.dma_start(wq_t[c][:], wq[c * 128:(c + 1) * 128, :])
            nc.sync.dma_start(wk_t[c][:], wk[c * 128:(c + 1) * 128, :])
            nc.sync.dma_start(xt[c][:], xT[c * 128:(c + 1) * 128, :])

        for c in range(CC):
            nc.sync.dma_start(wv_t[c][:], wv[c * 128:(c + 1) * 128, :])
        nc.sync.dma_start(bv_row[0:1, :], bv[0:1, :])
        for m in range(MT):
            nc.sync.dma_start(wp_t[m][:], wp[m * 128:(m + 1) * 128, :])
        nc.gpsimd.partition_broadcast(bv_bc[:], bv_row[0:1, :])
        for t in range(KB):
            nc.gpsimd.tensor_scalar(
                V2[t][:, :, 64:66],
                bv_bc[:, 0:12].rearrange("p (a b) -> p a b", a=HG),
                0.0, 1.0, mybir.AluOpType.mult, mybir.AluOpType.add)

        def qk_mtile(m):
            for wt, dst, bcol in ((wq_t, QT, m), (wk_t, KT, MT + m)):
                for n in range(NQ):
                    ps = psum.tile([128, 512], F32, tag="ps",
                                   name=f"psqk{m}_{n}")
                    for c in range(CC):
                        nc.tensor.matmul(
                            ps[:], wt[c][:, m * 128:(m + 1) * 128],
                            xt[c][:, n * 512:(n + 1) * 512],
                            start=(c == 0), stop=(c == CC - 1))
                    nc.vector.tensor_scalar_add(
                        dst[m][:, n * 512:(n + 1) * 512], ps[:],
                        bqk_t[:, bcol:bcol + 1])

        def v_tile(t):
            ps = psum_u.tile([128, CHG], F32, tag="psu", name=f"psv{t}")
            for c in range(CC):
                nc.tensor.matmul(ps[:], xt[c][:, t * 128:(t + 1) * 128],
                                 wv_t[c][:],
                                 start=(c == 0), stop=(c == CC - 1))
            nc.vector.tensor_add(
                V2[t][:, :, 0:64],
                ps[:].rearrange("p (h d) -> p h d", h=HG),
                bv_bc[:].rearrange("p (h d) -> p h d", h=HG))

        def v_proj(ts):
            for t in ts:
                v_tile(t)

        # ---- attention pieces ----
        def attn_pair(qs, j, k_hook=None):
            ha, hb = 2 * j, 2 * j + 1
            q0 = qs * 512
            psu = [psum_u.tile([128, 512], F32, tag="psu",
                               name=f"psu{j}_{qs}_{i}") for i in range(2)]

            def emit_pv(k, es):
                for i, h in enumerate((ha, hb)):
                    nc.tensor.matmul(
                        psu[i][0:66, :], V2[k][:, h, :],
                        es[:, i * 512:(i + 1) * 512],
                        start=(k == 0), stop=(k == KB - 1))

            # software pipeline: attnV lags the scores/exp by LAG k-steps
            # so the ~1.1us exp latency is not in the tensor-FIFO critical
            # path (attnV(k) sits between scores(k+LAG) and scores(k+LAG+1)).
            LAG = 2
            esq = []
            for k in range(KB):
                if k_hook is not None:
                    k_hook(k)
                pss = psum.tile([128, 1024], F32, tag="ps",
                                name=f"pss{j}_{qs}_{k}")
                # head a in PE rows 0-63 -> pss cols 0-511, head b in
                # rows 64-127 -> cols 512-1023; one shared PSUM tile so
                # the second matmul has no semaphore wait and the pair
                # runs concurrently in disjoint row groups.
                for i, off in ((0, 0), (1, 64)):
                    nc.tensor.matmul(
                        pss[:, i * 512:(i + 1) * 512],
                        KT[j][off:off + 64, k * 128:(k + 1) * 128],
                        QT[j][off:off + 64, q0:q0 + 512],
                        start=True, stop=True)
                es = spool.tile([128, 1024], BF16, tag="es",
                                name=f"es{j}_{qs}_{k}")
                if k % 16 in (1, 3, 5, 8, 10, 12, 14):
                    # DVE fast exp: bits = int16(S*A + B) -> bf16
                    nc.vector.tensor_scalar(
                        es[:].bitcast(I16), pss[:],
                        expc_t[:, 0:1], expc_t[:, 1:2],
                        mybir.AluOpType.mult, mybir.AluOpType.add)
                else:
                    nc.scalar.activation(es[:], pss[:], EXPF, scale=SCALE)
                esq.append(es)
                if k >= LAG:
                    emit_pv(k - LAG, esq[k - LAG])
            for k in range(KB - LAG, KB):
                emit_pv(k, esq[k])

            # denominators -> reciprocal -> replicate -> scale U rows
            for i, off in ((0, 0), (1, 64)):
                rb = rpool.tile([128, 512], F32, tag="rb",
                                name=f"rb{j}_{qs}_{i}")
                rc = rpool.tile([128, 512], F32, tag="rb",
                                name=f"rc{j}_{qs}_{i}")
                rg = rpool.tile([128, 8], F32, tag="rg",
                                name=f"rg{j}_{qs}_{i}")
                # denom row (512 on one partition) -> spread over 128
                # partitions so the exact reciprocal runs 4 elems/lane
                nc.vector.tensor_copy(rb[64:65, :], psu[i][64:65, :])
                nc.sync.dma_start(rg[:, 0:4], rb[64:65, :])
                nc.vector.reciprocal(rg[:, 4:8], rg[:, 0:4])
                nc.sync.dma_start(rc[0:1, :], rg[:, 4:8])
                nc.gpsimd.partition_broadcast(rc[0:64, :], rc[0:1, :])
                if off == 0:
                    nc.vector.tensor_mul(UT[j][0:64, q0:q0 + 512],
                                         psu[i][0:64, :], rc[0:64, :])
                else:
                    st = stpool.tile([64, 512], BF16, tag="st",
                                     name=f"st{j}_{qs}_{i}")
                    nc.vector.tensor_mul(st[0:64, :], psu[i][0:64, :],
                                         rc[0:64, :])
                    nc.sync.dma_start(UT[j][64:128, q0:q0 + 512],
                                      st[0:64, :])

        def proj_qs(qs):
            for qb in range(qs * 4, qs * 4 + 4):
                plo = psum_u.tile([128, 512], F32, tag="psu", name=f"pl{qb}")
                phi = psum_u.tile([128, 512], F32, tag="psu", name=f"ph{qb}")
                for m in range(MT):
                    lhsT = UT[m][:, qb * 128:(qb + 1) * 128]
                    nc.tensor.matmul(plo[:, 0:512], lhsT, wp_t[m][:, 0:512],
                                     start=(m == 0), stop=(m == MT - 1))
                    nc.tensor.matmul(phi[:, 0:256], lhsT,
                                     wp_t[m][:, 512:768],
                                     start=(m == 0), stop=(m == MT - 1))
                ot = opool.tile([128, C], BF16, tag="ost", name=f"ot{qb}")
                nc.scalar.copy(ot[:, 0:512], plo[:, 0:512])
                nc.scalar.copy(ot[:, 512:768], phi[:, 0:256])
                nc.sync.dma_start(out[qb * 128:(qb + 1) * 128, :], ot[:])

        # ---- emission schedule: pipeline QKV m-tiles into attention ----
        VHEAD = min(6, KB)
        qk_mtile(0)
        v_proj(range(VHEAD))
        attn_pair(0, 0, k_hook=lambda k: v_tile(k) if k >= VHEAD else None)
        qk_mtile(1)
        attn_pair(0, 1)
        qk_mtile(2)
        attn_pair(0, 2)
        for qs in range(1, NQ):
            attn_pair(qs, 0)
            proj_qs(qs - 1)
            attn_pair(qs, 1)
            attn_pair(qs, 2)
        proj_qs(NQ - 1)

    nc.compile()
    return nc


_built = {}


def _get_nc(n_tok=2048):
    if n_tok not in _built:
        _built[n_tok] = build(n_tok)
    return _built[n_tok]


def make_in_maps(x, Wqkv, bqkv, Wproj, exp_b=EXP_B):
    B, NT, _ = x.shape
    x = np.ascontiguousarray(np.asarray(x, dtype=np.float32))
    Wqkv = np.asarray(Wqkv, dtype=np.float32)
    bqkv = np.asarray(bqkv, dtype=np.float32)
    Wproj = np.asarray(Wproj, dtype=np.float32)
    expc = np.zeros((128, 2), dtype=np.float32)
    expc[:, 0] = EXP_A
    expc[:, 1] = exp_b
    in_maps = []
    for i in range(N_CORES):
        b, g = i // 2, i % 2
        s = g * CHG
        bq = bqkv[s:s + CHG].reshape(MT, 128).T
        bk = bqkv[C + s:C + s + CHG].reshape(MT, 128).T
        in_maps.append({
            "xT": np.ascontiguousarray(x[b].T.astype(BF)),
            "wq": np.ascontiguousarray(Wqkv[:, s:s + CHG].astype(BF)),
            "wk": np.ascontiguousarray(Wqkv[:, C + s:C + s + CHG].astype(BF)),
            "wv": np.ascontiguousarray(
                Wqkv[:, 2 * C + s:2 * C + s + CHG].astype(BF)),
            "wp": np.ascontiguousarray(Wproj[s:s + CHG, :].astype(BF)),
            "bqk": np.ascontiguousarray(
                np.concatenate([bq, bk], axis=1)).astype(np.float32),
            "bv": np.ascontiguousarray(
                bqkv[2 * C + s:2 * C + s + CHG][None, :]).astype(np.float32),
            "expc": expc,
        })
    return in_maps


def gather(results, bproj, B, NT):
    parts = [np.asarray(results[i]["out"], dtype=np.float32)
             for i in range(N_CORES)]
    out = np.stack([parts[2 * b] + parts[2 * b + 1] for b in range(B)])
    return (out + np.asarray(bproj, np.float32)[None, None, :]).astype(np.float32)


def kernel(x, Wqkv, bqkv, Wproj, bproj, _trace=False, _exp_b=EXP_B):
    x = np.asarray(x)
    B, NT, _ = x.shape
    nc = _get_nc(NT)
    in_maps = make_in_maps(x, Wqkv, bqkv, Wproj, exp_b=_exp_b)
    res = run_bass_kernel_spmd(nc, in_maps, core_ids=list(range(N_CORES)),
                               trace=_trace)
    out = gather(res.results, bproj, B, NT)
    if _trace:
        return out, res
    return out


# revision 12
# speedup vs baseline: 1.1722x; 1.0462x over previous
"""Multi-head attention block (12 heads, N=2048, C=768) on 8 NeuronCores.

Sharding: core i = (batch b = i//2, head-group g = i%2). Each core computes
attention for 6 heads of one batch plus its slice of the output projection
(row-sharded Wproj); the host sums the two head-group partials per batch.

Per-core dataflow (all matmuls bf16; fp32r runs 2cy/row on HW so bf16
operands ~halve QKV/out-proj tensor time):
  xT [768,2048] bf16 arrives host-transposed; QT/KT [384,2048] bf16 are
  column-major (head h lives at partitions (h%2)*64..+64 of tile h//2), V2
  is token-major with a ones column per head (66th col = pad).

  Heads are processed in pairs (a=2j at PE rows 0-63, b=2j+1 at rows 64-127).
  Per (pair, 512-query chunk qs, key block k):
    S^T_a -> pss[:, 0:512], S^T_b -> pss[:, 512:1024]  (two matmuls in
      disjoint PE row groups sharing one PSUM tile -> concurrent)
    es = exp(S/8) in ONE instruction for both heads: ACT exp for 3 of 4
      k-blocks; for k%4==3 the Vector engine computes a Schraudolph
      bit-trick exp (es_bits = int16(S*A + B), bitcast to bf16; A,B come
      from the `expc` input so they are calibratable without recompile).
      This splits the exp work (the ACT engine is otherwise the 2nd
      bottleneck at ~208us) at a ~0.9% U-error cost.
    U'_a += V2_a[k]^T @ es[:, 0:512], U'_b += V2_b[k]^T @ es[:, 512:1024]
      (PSUM [66,512] accumulated over k; row 64 = softmax denominator via
      the ones column; software-pipelined one k behind the scores)
  U rows are scaled by 1/denominator (DVE reciprocal + gpsimd partition
  broadcast + fused DVE multiply) into UT [384,2048] bf16; odd heads take
  a small DMA hop to land at partitions 64-127.
  out = UT^T-chunks @ Wproj_rows (bf16, PSUM-accumulated), DMA out.
"""

import numpy as np
import ml_dtypes
from contextlib import ExitStack

import concourse.bass as bass
import concourse.tile as tile
from concourse import bacc, mybir
from concourse.bass_utils import run_bass_kernel_spmd

N_CORES = 8
C = 768          # model dim
HG = 6           # heads per core
D = 64           # head dim
CHG = HG * D     # 384, per-group qkv width
CC = C // 128    # 6 contraction chunks
MT = CHG // 128  # 3 m-tiles for QT/KT
SCALE = 1.0 / 8.0

# Schraudolph fast-exp constants (bf16 bitcast):
#   es_bits = int16(S * EXP_A + EXP_B); bits reinterpreted as bf16
# EXP_A = 2^7 * log2(e) * SCALE; EXP_B = 127*2^7 - c with c fitted so the
# multiplicative error vs exp(S/8) has mean 1.0 (so ACT-exact and
# DVE-approx key blocks are mutually unbiased inside one softmax).
EXP_A = 128.0 * np.log2(np.e) * SCALE
EXP_B = 16248.72

F32 = mybir.dt.float32
BF16 = mybir.dt.bfloat16
I16 = mybir.dt.int16

BF = ml_dtypes.bfloat16


def build(n_tok: int = 2048):
    NT = n_tok
    KB = NT // 128           # key blocks
    NQ = NT // 512           # 512-wide query chunks
    EXPF = mybir.ActivationFunctionType.Exp

    nc = bacc.Bacc("TRN2", target_bir_lowering=False, debug=False,
                   num_devices=N_CORES)

    xT = nc.dram_tensor("xT", [C, NT], BF16, kind="ExternalInput").ap()
    wq = nc.dram_tensor("wq", [C, CHG], BF16, kind="ExternalInput").ap()
    wk = nc.dram_tensor("wk", [C, CHG], BF16, kind="ExternalInput").ap()
    wv = nc.dram_tensor("wv", [C, CHG], BF16, kind="ExternalInput").ap()
    wp = nc.dram_tensor("wp", [CHG, C], BF16, kind="ExternalInput").ap()
    bqk = nc.dram_tensor("bqk", [128, 2 * MT], F32, kind="ExternalInput").ap()
    bv = nc.dram_tensor("bv", [1, CHG], F32, kind="ExternalInput").ap()
    expc = nc.dram_tensor("expc", [128, 2], F32, kind="ExternalInput").ap()
    out = nc.dram_tensor("out", [NT, C], BF16, kind="ExternalOutput").ap()

    with tile.TileContext(nc) as tc, ExitStack() as ctx:
        wpool = ctx.enter_context(tc.tile_pool(name="w", bufs=1))
        perm = ctx.enter_context(tc.tile_pool(name="perm", bufs=1))
        # PSUM budget (8 banks): "ps" 3 x [128,1024] (6 banks) gives the
        # scores->exp pipeline depth 3 (so the ~1.1us exp latency is off the
        # critical path); "psu" 2 x [128,512] (2 banks) holds the attnV
        # accumulators of the in-flight head pair. v_tile and the output
        # projection borrow "ps"-pool tiles.
        psum = ctx.enter_context(tc.tile_pool(name="ps", bufs=3, space="PSUM"))
        psum_u = ctx.enter_context(tc.tile_pool(name="psu", bufs=2,
                                                space="PSUM"))

        # ---- persistent SBUF ----
        wq_t = [wpool.tile([128, CHG], BF16, tag=f"wq{c}", name=f"wq{c}")
                for c in range(CC)]
        wk_t = [wpool.tile([128, CHG], BF16, tag=f"wk{c}", name=f"wk{c}")
                for c in range(CC)]
        wv_t = [wpool.tile([128, CHG], BF16, tag=f"wv{c}", name=f"wv{c}")
                for c in range(CC)]
        wp_t = [wpool.tile([128, C], BF16, tag=f"wp{m}", name=f"wp{m}")
                for m in range(MT)]
        bqk_t = wpool.tile([128, 2 * MT], F32, tag="bqk")
        bv_row = wpool.tile([1, CHG], F32, tag="bvr")
        bv_bc = wpool.tile([128, CHG], F32, tag="bvb")
        expc_t = wpool.tile([128, 2], F32, tag="expc")

        QT = [perm.tile([128, NT], BF16, tag=f"qt{m}", name=f"qtt{m}")
              for m in range(MT)]
        KT = [perm.tile([128, NT], BF16, tag=f"kt{m}", name=f"ktt{m}")
              for m in range(MT)]
        V2 = [perm.tile([128, HG, 66], BF16, tag=f"v2{t}", name=f"v2t{t}")
              for t in range(KB)]
        UT = [perm.tile([128, NT], BF16, tag=f"ut{m}", name=f"utt{m}")
              for m in range(MT)]

        # ---- input DMA (interleaved per contraction chunk so qk_mtile(0)
        # matmul c unblocks as soon as wq[c]/wk[c]/xt[c] land) ----
        nc.sync.dma_start(bqk_t[:], bqk)
        nc.sync.dma_start(expc_t[:], expc)

        spool = ctx.enter_context(tc.tile_pool(name="es", bufs=14))
        rpool = ctx.enter_context(tc.tile_pool(name="rb", bufs=4))
        stpool = ctx.enter_context(tc.tile_pool(name="st", bufs=3))
        opool = ctx.enter_context(tc.tile_pool(name="ost", bufs=3))
        xpool = ctx.enter_context(tc.tile_pool(name="xt", bufs=1))

        # ---- QKV projection pieces ----
        xt = [xpool.tile([128, NT], BF16, tag=f"x{c}", name=f"xt{c}")
              for c in range(CC)]
        for c in range(CC):
            nc.sync.dma_start(wq_t[c][:], wq[c * 128:(c + 1) * 128, :])
            nc.sync.dma_start(wk_t[c][:], wk[c * 128:(c + 1) * 128, :])
            nc.sync.dma_start(xt[c][:], xT[c * 128:(c + 1) * 128, :])

        for c in range(CC):
            nc.sync.dma_start(wv_t[c][:], wv[c * 128:(c + 1) * 128, :])
        nc.sync.dma_start(bv_row[0:1, :], bv[0:1, :])
        for m in range(MT):
            nc.sync.dma_start(wp_t[m][:], wp[m * 128:(m + 1) * 128, :])
        nc.gpsimd.partition_broadcast(bv_bc[:], bv_row[0:1, :])
        for t in range(KB):
            nc.gpsimd.tensor_scalar(
                V2[t][:, :, 64:66],
                bv_bc[:, 0:12].rearrange("p (a b) -> p a b", a=HG),
                0.0, 1.0, mybir.AluOpType.mult, mybir.AluOpType.add)

        def qk_mtile(m):
            for wt, dst, bcol in ((wq_t, QT, m), (wk_t, KT, MT + m)):
                for n in range(NQ):
                    ps = psum.tile([128, 512], F32, tag="ps",
                                   name=f"psqk{m}_{n}")
                    for c in range(CC):
                        nc.tensor.matmul(
                            ps[:], wt[c][:, m * 128:(m + 1) * 128],
                            xt[c][:, n * 512:(n + 1) * 512],
                            start=(c == 0), stop=(c == CC - 1))
                    nc.vector.tensor_scalar_add(
                        dst[m][:, n * 512:(n + 1) * 512], ps[:],
                        bqk_t[:, bcol:bcol + 1])

        def v_tile(t):
            pv = psum.tile([128, 1024], F32, tag="ps", name=f"psv{t}")
            ps = pv[:, 0:CHG]
            for c in range(CC):
                nc.tensor.matmul(ps, xt[c][:, t * 128:(t + 1) * 128],
                                 wv_t[c][:],
                                 start=(c == 0), stop=(c == CC - 1))
            nc.vector.tensor_add(
                V2[t][:, :, 0:64],
                ps.rearrange("p (h d) -> p h d", h=HG),
                bv_bc[:].rearrange("p (h d) -> p h d", h=HG))

        def v_proj(ts):
            for t in ts:
                v_tile(t)

        # ---- attention pieces ----
        def attn_pair(qs, j, k_hook=None):
            ha, hb = 2 * j, 2 * j + 1
            q0 = qs * 512
            psu = [psum_u.tile([128, 512], F32, tag="psu",
                               name=f"psu{j}_{qs}_{i}") for i in range(2)]

            def emit_pv(k, es):
                for i, h in enumerate((ha, hb)):
                    nc.tensor.matmul(
                        psu[i][0:66, :], V2[k][:, h, :],
                        es[:, i * 512:(i + 1) * 512],
                        start=(k == 0), stop=(k == KB - 1))

            # software pipeline: attnV lags the scores/exp by LAG k-steps
            # so the ~1.1us exp latency is not in the tensor-FIFO critical
            # path (attnV(k) sits between scores(k+LAG) and scores(k+LAG+1)).
            LAG = 2
            esq = []
            for k in range(KB):
                if k_hook is not None:
                    k_hook(k)
                pss = psum.tile([128, 1024], F32, tag="ps",
                                name=f"pss{j}_{qs}_{k}")
                # head a in PE rows 0-63 -> pss cols 0-511, head b in
                # rows 64-127 -> cols 512-1023; one shared PSUM tile so
                # the second matmul has no semaphore wait and the pair
                # runs concurrently in disjoint row groups.
                for i, off in ((0, 0), (1, 64)):
                    nc.tensor.matmul(
                        pss[:, i * 512:(i + 1) * 512],
                        KT[j][off:off + 64, k * 128:(k + 1) * 128],
                        QT[j][off:off + 64, q0:q0 + 512],
                        start=True, stop=True)
                es = spool.tile([128, 1024], BF16, tag="es",
                                name=f"es{j}_{qs}_{k}")
                if k % 16 in (1, 3, 5, 8, 10, 12, 14):
                    # DVE fast exp: bits = int16(S*A + B) -> bf16
                    nc.vector.tensor_scalar(
                        es[:].bitcast(I16), pss[:],
                        expc_t[:, 0:1], expc_t[:, 1:2],
                        mybir.AluOpType.mult, mybir.AluOpType.add)
                else:
                    nc.scalar.activation(es[:], pss[:], EXPF, scale=SCALE)
                esq.append(es)
                if k >= LAG:
                    emit_pv(k - LAG, esq[k - LAG])
            for k in range(KB - LAG, KB):
                emit_pv(k, esq[k])

            # evacuate U'+denominator to SBUF right away (frees the psu
            # bank ~2us earlier than the old in-place path), then
            # denominators -> reciprocal -> replicate -> scale U rows
            for i, off in ((0, 0), (1, 64)):
                ur = rpool.tile([66, 512], F32, tag="ur",
                                name=f"ur{j}_{qs}_{i}")
                nc.vector.tensor_copy(ur[0:66, :], psu[i][0:66, :])
                rc = rpool.tile([128, 512], F32, tag="rb",
                                name=f"rc{j}_{qs}_{i}")
                rg = rpool.tile([128, 8], F32, tag="rg",
                                name=f"rg{j}_{qs}_{i}")
                # denom row (512 on one partition) -> spread over 128
                # partitions so the exact reciprocal runs 4 elems/lane
                nc.sync.dma_start(rg[:, 0:4], ur[64:65, :])
                nc.vector.reciprocal(rg[:, 4:8], rg[:, 0:4])
                nc.sync.dma_start(rc[0:1, :], rg[:, 4:8])
                nc.gpsimd.partition_broadcast(rc[0:64, :], rc[0:1, :])
                if off == 0:
                    nc.vector.tensor_mul(UT[j][0:64, q0:q0 + 512],
                                         ur[0:64, :], rc[0:64, :])
                else:
                    st = stpool.tile([64, 512], BF16, tag="st",
                                     name=f"st{j}_{qs}_{i}")
                    nc.vector.tensor_mul(st[0:64, :], ur[0:64, :],
                                         rc[0:64, :])
                    nc.sync.dma_start(UT[j][64:128, q0:q0 + 512],
                                      st[0:64, :])

        def proj_qs(qs):
            for qb in range(qs * 4, qs * 4 + 4):
                pt = psum.tile([128, 1024], F32, tag="ps", name=f"pt{qb}")
                for m in range(MT):
                    lhsT = UT[m][:, qb * 128:(qb + 1) * 128]
                    nc.tensor.matmul(pt[:, 0:512], lhsT, wp_t[m][:, 0:512],
                                     start=(m == 0), stop=(m == MT - 1))
                    nc.tensor.matmul(pt[:, 512:768], lhsT,
                                     wp_t[m][:, 512:768],
                                     start=(m == 0), stop=(m == MT - 1))
                ot = opool.tile([128, C], BF16, tag="ost", name=f"ot{qb}")
                nc.scalar.copy(ot[:, 0:512], pt[:, 0:512])
                nc.scalar.copy(ot[:, 512:768], pt[:, 512:768])
                nc.sync.dma_start(out[qb * 128:(qb + 1) * 128, :], ot[:])

        # ---- emission schedule: pipeline QKV m-tiles into attention ----
        VHEAD = min(6, KB)
        qk_mtile(0)
        v_proj(range(VHEAD))
        attn_pair(0, 0, k_hook=lambda k: v_tile(k) if k >= VHEAD else None)
        qk_mtile(1)
        attn_pair(0, 1)
        qk_mtile(2)
        attn_pair(0, 2)
        for qs in range(1, NQ):
            attn_pair(qs, 0)
            proj_qs(qs - 1)
            attn_pair(qs, 1)
            attn_pair(qs, 2)
        proj_qs(NQ - 1)

    nc.compile()
    return nc


_built = {}


def _get_nc(n_tok=2048):
    if n_tok not in _built:
        _built[n_tok] = build(n_tok)
    return _built[n_tok]


def make_in_maps(x, Wqkv, bqkv, Wproj, exp_b=EXP_B):
    B, NT, _ = x.shape
    x = np.ascontiguousarray(np.asarray(x, dtype=np.float32))
    Wqkv = np.asarray(Wqkv, dtype=np.float32)
    bqkv = np.asarray(bqkv, dtype=np.float32)
    Wproj = np.asarray(Wproj, dtype=np.float32)
    expc = np.zeros((128, 2), dtype=np.float32)
    expc[:, 0] = EXP_A
    expc[:, 1] = exp_b
    in_maps = []
    for i in range(N_CORES):
        b, g = i // 2, i % 2
        s = g * CHG
        bq = bqkv[s:s + CHG].reshape(MT, 128).T
        bk = bqkv[C + s:C + s + CHG].reshape(MT, 128).T
        in_maps.append({
            "xT": np.ascontiguousarray(x[b].T.astype(BF)),
            "wq": np.ascontiguousarray(Wqkv[:, s:s + CHG].astype(BF)),
            "wk": np.ascontiguousarray(Wqkv[:, C + s:C + s + CHG].astype(BF)),
            "wv": np.ascontiguousarray(
                Wqkv[:, 2 * C + s:2 * C + s + CHG].astype(BF)),
            "wp": np.ascontiguousarray(Wproj[s:s + CHG, :].astype(BF)),
            "bqk": np.ascontiguousarray(
                np.concatenate([bq, bk], axis=1)).astype(np.float32),
            "bv": np.ascontiguousarray(
                bqkv[2 * C + s:2 * C + s + CHG][None, :]).astype(np.float32),
            "expc": expc,
        })
    return in_maps


def gather(results, bproj, B, NT):
    parts = [np.asarray(results[i]["out"], dtype=np.float32)
             for i in range(N_CORES)]
    out = np.stack([parts[2 * b] + parts[2 * b + 1] for b in range(B)])
    return (out + np.asarray(bproj, np.float32)[None, None, :]).astype(np.float32)


def kernel(x, Wqkv, bqkv, Wproj, bproj, _trace=False, _exp_b=EXP_B):
    x = np.asarray(x)
    B, NT, _ = x.shape
    nc = _get_nc(NT)
    in_maps = make_in_maps(x, Wqkv, bqkv, Wproj, exp_b=_exp_b)
    res = run_bass_kernel_spmd(nc, in_maps, core_ids=list(range(N_CORES)),
                               trace=_trace)
    out = gather(res.results, bproj, B, NT)
    if _trace:
        return out, res
    return out


# revision 17
# speedup vs baseline: 1.2400x; 1.0578x over previous
"""Multi-head attention block (12 heads, N=2048, C=768) on 8 NeuronCores.

Sharding: core i = (batch b = i//2, head-group g = i%2). Each core computes
attention for 6 heads of one batch plus its slice of the output projection
(row-sharded Wproj); the host sums the two head-group partials per batch.

Per-core dataflow (all matmuls bf16; fp32r runs 2cy/row on HW so bf16
operands ~halve QKV/out-proj tensor time):
  xT [768,2048] bf16 arrives host-transposed; QT/KT [384,2048] bf16 are
  column-major (head h lives at partitions (h%2)*64..+64 of tile h//2), V2
  is token-major with a ones column per head (66th col = pad).

  Heads are processed in pairs (a=2j at PE rows 0-63, b=2j+1 at rows 64-127).
  Per (pair, 512-query chunk qs, key block k):
    S^T_a -> pss[:, 0:512], S^T_b -> pss[:, 512:1024]  (two matmuls in
      disjoint PE row groups sharing one PSUM tile -> concurrent)
    es = exp(S/8) in ONE instruction for both heads: ACT exp for 3 of 4
      k-blocks; for k%4==3 the Vector engine computes a Schraudolph
      bit-trick exp (es_bits = int16(S*A + B), bitcast to bf16; A,B come
      from the `expc` input so they are calibratable without recompile).
      This splits the exp work (the ACT engine is otherwise the 2nd
      bottleneck at ~208us) at a ~0.9% U-error cost.
    U'_a += V2_a[k]^T @ es[:, 0:512], U'_b += V2_b[k]^T @ es[:, 512:1024]
      (PSUM [66,512] accumulated over k; row 64 = softmax denominator via
      the ones column; software-pipelined one k behind the scores)
  U rows are scaled by 1/denominator (DVE reciprocal + gpsimd partition
  broadcast + fused DVE multiply) into UT [384,2048] bf16; odd heads take
  a small DMA hop to land at partitions 64-127.
  out = UT^T-chunks @ Wproj_rows (bf16, PSUM-accumulated), DMA out.
"""

import numpy as np
import ml_dtypes
from contextlib import ExitStack

import concourse.bass as bass
import concourse.tile as tile
from concourse import bacc, mybir
from concourse.bass_utils import run_bass_kernel_spmd

N_CORES = 8
C = 768          # model dim
HG = 6           # heads per core
D = 64           # head dim
CHG = HG * D     # 384, per-group qkv width
CC = C // 128    # 6 contraction chunks
MT = CHG // 128  # 3 m-tiles for QT/KT
SCALE = 1.0 / 8.0

# Schraudolph fast-exp constants (bf16 bitcast):
#   es_bits = int16(S * EXP_A + EXP_B); bits reinterpreted as bf16
# EXP_A = 2^7 * log2(e) * SCALE; EXP_B = 127*2^7 - c with c fitted so the
# multiplicative error vs exp(S/8) has mean 1.0 (so ACT-exact and
# DVE-approx key blocks are mutually unbiased inside one softmax).
EXP_A = 128.0 * np.log2(np.e) * SCALE
EXP_B = 16248.72

F32 = mybir.dt.float32
BF16 = mybir.dt.bfloat16
I16 = mybir.dt.int16

BF = ml_dtypes.bfloat16


def build(n_tok: int = 2048):
    NT = n_tok
    KB = NT // 128           # key blocks
    NQ = NT // 512           # 512-wide query chunks
    EXPF = mybir.ActivationFunctionType.Exp

    nc = bacc.Bacc("TRN2", target_bir_lowering=False, debug=False,
                   num_devices=N_CORES)

    xT = nc.dram_tensor("xT", [C, NT], BF16, kind="ExternalInput").ap()
    wq = nc.dram_tensor("wq", [C, CHG], BF16, kind="ExternalInput").ap()
    wk = nc.dram_tensor("wk", [C, CHG], BF16, kind="ExternalInput").ap()
    wv = nc.dram_tensor("wv", [C, CHG], BF16, kind="ExternalInput").ap()
    wp = nc.dram_tensor("wp", [CHG, C], BF16, kind="ExternalInput").ap()
    bqk = nc.dram_tensor("bqk", [128, 2 * MT], F32, kind="ExternalInput").ap()
    bv = nc.dram_tensor("bv", [1, CHG], F32, kind="ExternalInput").ap()
    expc = nc.dram_tensor("expc", [128, 2], F32, kind="ExternalInput").ap()
    out = nc.dram_tensor("out", [NT, C], BF16, kind="ExternalOutput").ap()

    with tile.TileContext(nc) as tc, ExitStack() as ctx:
        wpool = ctx.enter_context(tc.tile_pool(name="w", bufs=1))
        perm = ctx.enter_context(tc.tile_pool(name="perm", bufs=1))
        # PSUM budget (8 banks): "ps" 3 x [128,1024] (6 banks) gives the
        # scores->exp pipeline depth 3 (so the ~1.1us exp latency is off the
        # critical path); "psu" 2 x [128,512] (2 banks) holds the attnV
        # accumulators of the in-flight head pair. v_tile and the output
        # projection borrow "ps"-pool tiles.
        psum = ctx.enter_context(tc.tile_pool(name="ps", bufs=3, space="PSUM"))
        psum_u = ctx.enter_context(tc.tile_pool(name="psu", bufs=2,
                                                space="PSUM"))

        # ---- persistent SBUF ----
        wq_t = [wpool.tile([128, CHG], BF16, tag=f"wq{c}", name=f"wq{c}")
                for c in range(CC)]
        wk_t = [wpool.tile([128, CHG], BF16, tag=f"wk{c}", name=f"wk{c}")
                for c in range(CC)]
        wv_t = [wpool.tile([128, CHG], BF16, tag=f"wv{c}", name=f"wv{c}")
                for c in range(CC)]
        wp_t = [wpool.tile([128, C], BF16, tag=f"wp{m}", name=f"wp{m}")
                for m in range(MT)]
        bqk_t = wpool.tile([128, 2 * MT], F32, tag="bqk")
        bv_row = wpool.tile([1, CHG], F32, tag="bvr")
        bv_bc = wpool.tile([128, CHG], F32, tag="bvb")
        expc_t = wpool.tile([128, 2], F32, tag="expc")

        QT = [perm.tile([128, NT], BF16, tag=f"qt{m}", name=f"qtt{m}")
              for m in range(MT)]
        KT = [perm.tile([128, NT], BF16, tag=f"kt{m}", name=f"ktt{m}")
              for m in range(MT)]
        # V2 per-head stride padded to 128 cols so the attnV stationary is a
        # full 128-col weight load -> FWL (fast weight load) halves LDWEIGHTS
        V2 = [perm.tile([128, HG, 128], BF16, tag=f"v2{t}", name=f"v2t{t}")
              for t in range(KB)]
        UT = [perm.tile([128, NT], BF16, tag=f"ut{m}", name=f"utt{m}")
              for m in range(MT)]

        # ---- input DMA (interleaved per contraction chunk so qk_mtile(0)
        # matmul c unblocks as soon as wq[c]/wk[c]/xt[c] land) ----
        nc.sync.dma_start(bqk_t[:], bqk)
        nc.sync.dma_start(expc_t[:], expc)

        spool = ctx.enter_context(tc.tile_pool(name="es", bufs=14))
        rpool = ctx.enter_context(tc.tile_pool(name="rb", bufs=4))
        stpool = ctx.enter_context(tc.tile_pool(name="st", bufs=3))
        opool = ctx.enter_context(tc.tile_pool(name="ost", bufs=3))
        xpool = ctx.enter_context(tc.tile_pool(name="xt", bufs=1))

        # ---- QKV projection pieces ----
        xt = [xpool.tile([128, NT], BF16, tag=f"x{c}", name=f"xt{c}")
              for c in range(CC)]
        for c in range(CC):
            nc.sync.dma_start(wq_t[c][:], wq[c * 128:(c + 1) * 128, :])
            nc.sync.dma_start(wk_t[c][:], wk[c * 128:(c + 1) * 128, :])
            nc.sync.dma_start(xt[c][:], xT[c * 128:(c + 1) * 128, :])

        for c in range(CC):
            nc.sync.dma_start(wv_t[c][:], wv[c * 128:(c + 1) * 128, :])
        nc.sync.dma_start(bv_row[0:1, :], bv[0:1, :])
        for m in range(MT):
            nc.sync.dma_start(wp_t[m][:], wp[m * 128:(m + 1) * 128, :])
        nc.gpsimd.partition_broadcast(bv_bc[:], bv_row[0:1, :])
        for t in range(KB):
            nc.gpsimd.tensor_scalar(
                V2[t][:, :, 64:66],
                bv_bc[:, 0:12].rearrange("p (a b) -> p a b", a=HG),
                0.0, 1.0, mybir.AluOpType.mult, mybir.AluOpType.add)
            nc.gpsimd.memset(V2[t][:, :, 66:128], 0)

        def qk_group(m, n, which):
            wt, dst, bcol = ((wq_t, QT, m) if which == "q"
                             else (wk_t, KT, MT + m))
            ps = psum.tile([128, 512], F32, tag="ps",
                           name=f"psqk{which}{m}_{n}")
            for c in range(CC):
                nc.tensor.matmul(
                    ps[:], wt[c][:, m * 128:(m + 1) * 128],
                    xt[c][:, n * 512:(n + 1) * 512],
                    start=(c == 0), stop=(c == CC - 1))
            nc.vector.tensor_scalar_add(
                dst[m][:, n * 512:(n + 1) * 512], ps[:],
                bqk_t[:, bcol:bcol + 1])

        def v_tile(t):
            pv = psum.tile([128, 1024], F32, tag="ps", name=f"psv{t}")
            ps = pv[:, 0:CHG]
            for c in range(CC):
                nc.tensor.matmul(ps, xt[c][:, t * 128:(t + 1) * 128],
                                 wv_t[c][:],
                                 start=(c == 0), stop=(c == CC - 1))
            nc.vector.tensor_add(
                V2[t][:, :, 0:64],
                ps.rearrange("p (h d) -> p h d", h=HG),
                bv_bc[:].rearrange("p (h d) -> p h d", h=HG))

        def v_proj(ts):
            for t in ts:
                v_tile(t)

        # ---- attention pieces ----
        def attn_pair(qs, j, k_hook=None):
            ha, hb = 2 * j, 2 * j + 1
            q0 = qs * 512
            psu = [psum_u.tile([128, 512], F32, tag="psu",
                               name=f"psu{j}_{qs}_{i}") for i in range(2)]

            def emit_pv(k, es):
                for i, h in enumerate((ha, hb)):
                    nc.tensor.matmul(
                        psu[i][:, :], V2[k][:, h, :],
                        es[:, i * 512:(i + 1) * 512],
                        start=(k == 0), stop=(k == KB - 1))

            def emit_scores(k):
                pss = psum.tile([128, 1024], F32, tag="ps",
                                name=f"pss{j}_{qs}_{k}")
                # head a in PE rows 0-63 -> pss cols 0-511, head b in
                # rows 64-127 -> cols 512-1023; one shared PSUM tile so
                # the second matmul has no semaphore wait and the pair
                # runs concurrently in disjoint row groups.
                for i, off in ((0, 0), (1, 64)):
                    nc.tensor.matmul(
                        pss[:, i * 512:(i + 1) * 512],
                        KT[j][off:off + 64, k * 128:(k + 1) * 128],
                        QT[j][off:off + 64, q0:q0 + 512],
                        start=True, stop=True)
                es = spool.tile([128, 1024], BF16, tag="es",
                                name=f"es{j}_{qs}_{k}")
                if k % 2 == 1 and k != KB - 1:
                    # DVE fast exp: bits = int16(S*A + B) -> bf16
                    nc.vector.tensor_scalar(
                        es[:].bitcast(I16), pss[:],
                        expc_t[:, 0:1], expc_t[:, 1:2],
                        mybir.AluOpType.mult, mybir.AluOpType.add)
                else:
                    nc.scalar.activation(es[:], pss[:], EXPF, scale=SCALE)
                return es

            # software pipeline: emit scores/exp for a PAIR of k-steps, then
            # the attnV matmuls lagging LAG k-steps behind. The lag keeps the
            # ~1.1us exp latency out of the tensor-FIFO critical path, and
            # the 2-step grouping halves the scores<->attnV row-group
            # boundary overhead.
            LAG = 4
            esq = []
            for kk in range(0, KB, 2):
                if k_hook is not None:
                    k_hook(kk)
                esq.append(emit_scores(kk))
                esq.append(emit_scores(kk + 1))
                for k in (kk - LAG, kk - LAG + 1):
                    if k >= 0:
                        emit_pv(k, esq[k])
            for k in range(max(0, KB - LAG), KB):
                emit_pv(k, esq[k])

            # evacuate U'+denominator to SBUF right away (frees the psu
            # bank ~2us earlier than the old in-place path), then
            # denominators -> reciprocal -> replicate -> scale U rows
            for i, off in ((0, 0), (1, 64)):
                ur = rpool.tile([66, 512], F32, tag="ur",
                                name=f"ur{j}_{qs}_{i}")
                nc.vector.tensor_copy(ur[0:66, :], psu[i][0:66, :])
                rc = rpool.tile([128, 512], F32, tag="rb",
                                name=f"rc{j}_{qs}_{i}")
                rg = rpool.tile([128, 8], F32, tag="rg",
                                name=f"rg{j}_{qs}_{i}")
                # denom row (512 on one partition) -> spread over 128
                # partitions so the exact reciprocal runs 4 elems/lane
                nc.sync.dma_start(rg[:, 0:4], ur[64:65, :])
                nc.vector.reciprocal(rg[:, 4:8], rg[:, 0:4])
                nc.sync.dma_start(rc[0:1, :], rg[:, 4:8])
                nc.gpsimd.partition_broadcast(rc[0:64, :], rc[0:1, :])
                if off == 0:
                    nc.vector.tensor_mul(UT[j][0:64, q0:q0 + 512],
                                         ur[0:64, :], rc[0:64, :])
                else:
                    st = stpool.tile([64, 512], BF16, tag="st",
                                     name=f"st{j}_{qs}_{i}")
                    nc.vector.tensor_mul(st[0:64, :], ur[0:64, :],
                                         rc[0:64, :])
                    nc.sync.dma_start(UT[j][64:128, q0:q0 + 512],
                                      st[0:64, :])

        def proj_qs(qs):
            for qb in range(qs * 4, qs * 4 + 4):
                pt = psum.tile([128, 1024], F32, tag="ps", name=f"pt{qb}")
                for m in range(MT):
                    lhsT = UT[m][:, qb * 128:(qb + 1) * 128]
                    nc.tensor.matmul(pt[:, 0:512], lhsT, wp_t[m][:, 0:512],
                                     start=(m == 0), stop=(m == MT - 1))
                    nc.tensor.matmul(pt[:, 512:768], lhsT,
                                     wp_t[m][:, 512:768],
                                     start=(m == 0), stop=(m == MT - 1))
                ot = opool.tile([128, C], BF16, tag="ost", name=f"ot{qb}")
                nc.scalar.copy(ot[:, 0:512], pt[:, 0:512])
                nc.scalar.copy(ot[:, 512:768], pt[:, 512:768])
                nc.sync.dma_start(out[qb * 128:(qb + 1) * 128, :], ot[:])

        # ---- emission schedule ----
        # Prologue: K m-tile 0 (all n) + Q m-tile 0 (n=0 only) + first V
        # tiles -- the minimum for attn_pair(0,0) to run. Everything else
        # (remaining V tiles, K/Q m-tiles 1-2, and Q n-chunks deferred until
        # the qs that reads them) is queued as "pieces" drip-fed into the
        # attention loop via k_hook, filling tensor slack left by exp
        # latency instead of running as serial blocks.
        VHEAD = min(6, KB)
        pieces = []
        for t in range(VHEAD, KB):
            pieces.append(lambda t=t: v_tile(t))
        for m in (1, 2):
            for n in range(NQ):
                pieces.append(lambda m=m, n=n: qk_group(m, n, "k"))
            pieces.append(lambda m=m: qk_group(m, 0, "q"))
        for n in range(1, NQ):
            for m in range(MT):
                pieces.append(lambda m=m, n=n: qk_group(m, n, "q"))

        def hook(kk):
            for _ in range(2):
                if pieces:
                    pieces.pop(0)()

        for n in range(NQ):
            qk_group(0, n, "k")
        qk_group(0, 0, "q")
        v_proj(range(VHEAD))
        attn_pair(0, 0, k_hook=hook)
        attn_pair(0, 1, k_hook=hook)
        attn_pair(0, 2, k_hook=hook)
        for qs in range(1, NQ):
            attn_pair(qs, 0, k_hook=hook)
            proj_qs(qs - 1)
            attn_pair(qs, 1, k_hook=hook)
            attn_pair(qs, 2, k_hook=hook)
        proj_qs(NQ - 1)

    nc.compile()
    return nc


_built = {}


def _get_nc(n_tok=2048):
    if n_tok not in _built:
        _built[n_tok] = build(n_tok)
    return _built[n_tok]


def make_in_maps(x, Wqkv, bqkv, Wproj, exp_b=EXP_B):
    B, NT, _ = x.shape
    x = np.ascontiguousarray(np.asarray(x, dtype=np.float32))
    Wqkv = np.asarray(Wqkv, dtype=np.float32)
    bqkv = np.asarray(bqkv, dtype=np.float32)
    Wproj = np.asarray(Wproj, dtype=np.float32)
    expc = np.zeros((128, 2), dtype=np.float32)
    expc[:, 0] = EXP_A
    expc[:, 1] = exp_b
    in_maps = []
    for i in range(N_CORES):
        b, g = i // 2, i % 2
        s = g * CHG
        bq = bqkv[s:s + CHG].reshape(MT, 128).T
        bk = bqkv[C + s:C + s + CHG].reshape(MT, 128).T
        in_maps.append({
            "xT": np.ascontiguousarray(x[b].T.astype(BF)),
            "wq": np.ascontiguousarray(Wqkv[:, s:s + CHG].astype(BF)),
            "wk": np.ascontiguousarray(Wqkv[:, C + s:C + s + CHG].astype(BF)),
            "wv": np.ascontiguousarray(
                Wqkv[:, 2 * C + s:2 * C + s + CHG].astype(BF)),
            "wp": np.ascontiguousarray(Wproj[s:s + CHG, :].astype(BF)),
            "bqk": np.ascontiguousarray(
                np.concatenate([bq, bk], axis=1)).astype(np.float32),
            "bv": np.ascontiguousarray(
                bqkv[2 * C + s:2 * C + s + CHG][None, :]).astype(np.float32),
            "expc": expc,
        })
    return in_maps


def gather(results, bproj, B, NT):
    parts = [np.asarray(results[i]["out"], dtype=np.float32)
             for i in range(N_CORES)]
    out = np.stack([parts[2 * b] + parts[2 * b + 1] for b in range(B)])
    return (out + np.asarray(bproj, np.float32)[None, None, :]).astype(np.float32)


def kernel(x, Wqkv, bqkv, Wproj, bproj, _trace=False, _exp_b=EXP_B):
    x = np.asarray(x)
    B, NT, _ = x.shape
    nc = _get_nc(NT)
    in_maps = make_in_maps(x, Wqkv, bqkv, Wproj, exp_b=_exp_b)
    res = run_bass_kernel_spmd(nc, in_maps, core_ids=list(range(N_CORES)),
                               trace=_trace)
    out = gather(res.results, bproj, B, NT)
    if _trace:
        return out, res
    return out
